# revision 1
# baseline (speedup 1.0000x reference)
"""Trainium2 Bass kernel for the CCG supertagger BERT model.

Data-parallel over batch: 16 samples -> 8 cores x 2 samples.
Activations kept transposed [H (6 chunks of 128), T=512 tokens] in SBUF.
fp32r matmuls for the residual-stream projections; bf16 for attention
internals / Wo2 / head-w2 (fp32->bf16 cast done inside gpsimd DMA).
"""
import numpy as np

import concourse.bass as bass
import concourse.tile as tile
from concourse import bacc, mybir
from concourse.bass_utils import run_bass_kernel_spmd
from concourse.masks import make_identity

F32 = mybir.dt.float32
F32R = mybir.dt.float32r
BF16 = mybir.dt.bfloat16
I32 = mybir.dt.int32
AF = mybir.ActivationFunctionType
ALU = mybir.AluOpType

B, S, W = 16, 256, 128
V, H, L, NH, DH, FF, C = 30522, 768, 12, 12, 64, 3072, 425
EPS = 1e-12
N_CORES = 8
BPC = B // N_CORES          # samples per core
T = BPC * S                 # tokens per core (512)
HC = H // 128               # 6
FFC = FF // 128             # 24
TC = T // 128               # 4 token chunks
M1 = 1024
M1C = M1 // 128             # 8
CPAD = 448                  # padded class dim for sbuf tiles


DEBUG_TAPS = False


def build_program(n_layers=L):
    nc = bacc.Bacc("TRN2", target_bir_lowering=False, debug=False,
                   num_devices=N_CORES)

    dt_ = lambda name, shape, dt, kind: nc.dram_tensor(name, shape, dt, kind=kind).ap()
    # per-core sharded inputs
    enc = dt_("enc", [T, 1], I32, "ExternalInput")
    ab = dt_("ab", [T], F32, "ExternalInput")            # attn bias (per key pos)
    pmat = dt_("pmat", [T, 128], F32, "ExternalInput")   # pooling matrices
    # replicated model inputs
    word_emb = dt_("word_emb", [V, H], F32, "ExternalInput")
    pos_emb = dt_("pos_emb", [S, H], F32, "ExternalInput")
    type_emb = dt_("type_emb", [1, H], F32, "ExternalInput")
    emb_ln_s = dt_("emb_ln_s", [H], F32, "ExternalInput")
    emb_ln_b = dt_("emb_ln_b", [H], F32, "ExternalInput")
    Wq = dt_("Wq", [L, H, H], F32, "ExternalInput")
    bq = dt_("bq", [L, H], F32, "ExternalInput")
    Wk = dt_("Wk", [L, H, H], F32, "ExternalInput")
    bk = dt_("bk", [L, H], F32, "ExternalInput")
    Wv = dt_("Wv", [L, H, H], F32, "ExternalInput")
    bv = dt_("bv", [L, H], F32, "ExternalInput")
    Wo = dt_("Wo", [L, H, H], F32, "ExternalInput")
    bo = dt_("bo", [L, H], F32, "ExternalInput")
    ln1_s = dt_("ln1_s", [L, H], F32, "ExternalInput")
    ln1_b = dt_("ln1_b", [L, H], F32, "ExternalInput")
    Wi = dt_("Wi", [L, H, FF], F32, "ExternalInput")
    bi = dt_("bi", [L, FF], F32, "ExternalInput")
    Wo2 = dt_("Wo2", [L, FF, H], F32, "ExternalInput")
    bo2 = dt_("bo2", [L, H], F32, "ExternalInput")
    ln2_s = dt_("ln2_s", [L, H], F32, "ExternalInput")
    ln2_b = dt_("ln2_b", [L, H], F32, "ExternalInput")
    w1 = dt_("w1", [H, M1], F32, "ExternalInput")
    b1 = dt_("b1", [M1], F32, "ExternalInput")
    w2 = dt_("w2", [M1, C], F32, "ExternalInput")
    b2 = dt_("b2", [C], F32, "ExternalInput")
    cones = dt_("cones", [128, 1], F32, "ExternalInput")   # column of ones
    crow = dt_("crow", [1, 128], F32, "ExternalInput")     # row of ones
    ceps = dt_("ceps", [128, 1], F32, "ExternalInput")     # EPS constant
    out_d = dt_("out", [T, C], F32, "ExternalOutput")
    if DEBUG_TAPS:
        dbg_x0 = dt_("dbg_x0", [H, T], F32, "ExternalOutput")
        dbg_q = dt_("dbg_q", [H, T], BF16, "ExternalOutput")
        dbg_ctx = dt_("dbg_ctx", [T, H], BF16, "ExternalOutput")
        dbg_a = dt_("dbg_a", [H, T], F32, "ExternalOutput")
        dbg_h2 = dt_("dbg_h2", [H, T], F32, "ExternalOutput")
        dbg_f2 = dt_("dbg_f2", [H, T], F32, "ExternalOutput")
        dbg_x1 = dt_("dbg_x1", [H, T], F32, "ExternalOutput")
        dbg_exp0 = dt_("dbg_exp0", [128, 256], BF16, "ExternalOutput")
        dbg_exp1 = dt_("dbg_exp1", [128, 256], BF16, "ExternalOutput")
        dbg_rec0 = dt_("dbg_rec0", [128, 1], F32, "ExternalOutput")
        dbg_v = dt_("dbg_v", [T, H], BF16, "ExternalOutput")

    with tile.TileContext(nc) as tc:
        _emit(nc, tc, n_layers, locals())
    nc.compile()
    return nc


def _emit(nc, tc, n_layers, d):
    from contextlib import ExitStack
    ctx = ExitStack()
    with ctx:
        _emit_body(nc, tc, n_layers, d, ctx)


def _emit_body(nc, tc, n_layers, d, ctx):
    pool = lambda name, bufs, space="SBUF": ctx.enter_context(
        tc.tile_pool(name=name, bufs=bufs, space=space))

    p_xt = pool("xt", 3)          # [128, HC, 512] f32r residual-stream acts
    p_xb = pool("xb", 2)          # [128, HC, 512] bf16 rhs copies (Xb, H2b)
    p_qk = pool("qk", 3)          # [128, HC, 512] bf16 (QT, KT, VT/ctxT, reluT)
    p_v = pool("v", 1)            # [128, TC, 768] bf16 token-major V
    p_ctx = pool("ctxp", 1)       # [128, TC, 768] bf16 token-major ctx
    p_exp = pool("exp", 4)        # [128, 256] bf16 exp tiles
    p_scr = pool("scr", 3)        # [128, 768] f32 scratch
    p_gel = pool("gel", 1)        # [128, 6, 512] bf16 gelu quarter
    p_f2a = pool("f2a", 1)        # [128, HC, 512] f32 FFN accum / emb x0 / w1 / f2sb
    p_w6 = pool("w6", 2)          # [128, HC, 768] bf16 whole QKVO weights
    p_wi = pool("wi", 2)          # [128, HC, 768] bf16 Wi quarters / head w2
    p_wb = pool("wb", 8)          # [128, 768] bf16 Wo2 k-bands
    p_f1 = pool("f1", 1)          # [128, M1C, 512] bf16 head f1relu
    p_bias = pool("bias", 6)      # [128, 24] f32 per-partition bias/scale tiles
    p_vec = pool("vec", 3)        # [1, 512] f32 LN stat vectors
    p_vec2 = pool("vec2", 1)      # [1, 1024] f32 (rstd | -mu*rstd)
    p_lnbc = pool("lnbc", 2)      # [128, 1024] f32 broadcast LN stats / b2bc
    p_dram = pool("dram", 2, "DRAM")
    p_sm = pool("sm", 2)          # small per-chunk scalars
    p_cst = pool("cst", 1)        # constants
    p_pos = pool("pos", 1)

    ps_mm = pool("ps_mm", 3, "PSUM")    # [128, 512]
    ps_sc = pool("ps_sc", 2, "PSUM")    # [128, 256] scores / [1, 512] LN stats
    ps_cx = pool("ps_cx", 2, "PSUM")    # [128, 64] ctx
    ps_su = pool("ps_su", 1, "PSUM")    # [128, 1] softmax sums

    enc, ab, pmat = d["enc"], d["ab"], d["pmat"]
    word_emb, pos_emb, type_emb = d["word_emb"], d["pos_emb"], d["type_emb"]
    emb_ln_s, emb_ln_b = d["emb_ln_s"], d["emb_ln_b"]
    out_d = d["out_d"]

    # ---- constants ----
    ident = p_cst.tile([128, 128], BF16, tag="ident")
    make_identity(nc, ident[:])
    identf = p_cst.tile([128, 128], F32, tag="identf")
    make_identity(nc, identf[:])
    ones_c = p_cst.tile([128, 1], F32R, tag="ones_c")
    nc.sync.dma_start(ones_c[:], d["cones"][:].bitcast(F32R))
    ones_cb = p_cst.tile([128, 1], BF16, tag="ones_cb")
    nc.gpsimd.dma_start(ones_cb[:], d["cones"][:])
    ones_r = p_cst.tile([1, 128], F32R, tag="ones_r")
    nc.sync.dma_start(ones_r[:], d["crow"][:].bitcast(F32R))
    eps_t = p_cst.tile([128, 1], F32, tag="eps")
    nc.sync.dma_start(eps_t[:], d["ceps"][:])

    # attn bias as [128, TC]
    ab_t = p_cst.tile([128, TC], F32, tag="ab")
    nc.sync.dma_start(ab_t[:], ab.rearrange("(c p) -> p c", p=128))

    def ln_pair(ap_s, ap_b, tag):
        t = p_bias.tile([128, 2 * HC], F32, tag="bias")
        nc.sync.dma_start(t[:, 0:HC], ap_s.rearrange("(c p) -> p c", p=128))
        nc.sync.dma_start(t[:, HC:2 * HC], ap_b.rearrange("(c p) -> p c", p=128))
        return t

    # =============== embedding ===============
    x0 = p_f2a.tile([128, TC, H], F32, tag="f2a")
    for c in range(TC):
        idx_t = p_sm.tile([128, 1], I32, tag="idx")
        nc.sync.dma_start(idx_t[:], enc[128 * c:128 * (c + 1), :])
        nc.gpsimd.indirect_dma_start(
            out=x0[:, c, :], out_offset=None, in_=word_emb[:],
            in_offset=bass.IndirectOffsetOnAxis(ap=idx_t[:, :1], axis=0))
    pos_t = p_pos.tile([128, 2, H], F32, tag="pos")
    nc.sync.dma_start(pos_t[:, 0, :], pos_emb[0:128, :])
    nc.sync.dma_start(pos_t[:, 1, :], pos_emb[128:256, :])
    typ_t = p_pos.tile([128, H], F32, tag="typ")
    nc.sync.dma_start(typ_t[:], type_emb[0:1, :].partition_broadcast(128)[:, 0, :])

    emb_sb = ln_pair(emb_ln_s, emb_ln_b, "embln")
    X = p_xt.tile([128, HC, T], F32R, tag="xt")
    for c in range(TC):
        xc = x0[:, c, :]
        nc.vector.tensor_tensor(out=xc, in0=xc, in1=pos_t[:, c % 2, :], op=ALU.add)
        nc.vector.tensor_tensor(out=xc, in0=xc, in1=typ_t[:], op=ALU.add)
        # layernorm over free dim (token-major)
        su = p_sm.tile([128, 4], F32, tag="stat")
        nc.vector.reduce_sum(out=su[:, 0:1], in_=xc, axis=mybir.AxisListType.X)
        sq = p_scr.tile([128, H], F32, tag="scr")
        nc.scalar.activation(sq[:], xc, AF.Square, accum_out=su[:, 1:2])
        st = p_sm.tile([128, 4], F32, tag="stat2")
        nc.vector.tensor_scalar_mul(st[:, 0:1], su[:, 0:1], 1.0 / H)      # mu
        nc.vector.tensor_scalar_mul(st[:, 1:2], su[:, 1:2], 1.0 / H)      # m2
        nc.vector.tensor_tensor(out=st[:, 2:3], in0=st[:, 0:1], in1=st[:, 0:1], op=ALU.mult)
        nc.vector.tensor_tensor(out=st[:, 3:4], in0=st[:, 1:2], in1=st[:, 2:3], op=ALU.subtract)
        sd = p_sm.tile([128, 2], F32, tag="stat3")
        nc.scalar.activation(sd[:, 0:1], st[:, 3:4], AF.Sqrt, bias=eps_t[:, 0:1])
        nc.vector.reciprocal(sd[:, 1:2], sd[:, 0:1])
        # x_hat = (x - mu) * rstd
        nc.vector.tensor_scalar(out=xc, in0=xc, scalar1=st[:, 0:1], scalar2=sd[:, 1:2],
                                op0=ALU.subtract, op1=ALU.mult)
        # transpose into X_T (fp32), fused scale/bias
        for k in range(HC):
            pt = ps_mm.tile([128, 128], F32, tag="ps_mm")
            nc.tensor.transpose(pt[:], xc[:, 128 * k:128 * (k + 1)], identf[:])
            nc.scalar.activation(X[:, k, 128 * c:128 * (c + 1)], pt[:],
                                 AF.Identity, scale=emb_sb[:, k:k + 1],
                                 bias=emb_sb[:, HC + k:HC + k + 1])

    def tap(name, tile_, nchunks, dt=F32):
        if not DEBUG_TAPS or name not in d:
            return
        ap = d[name]
        for k in range(nchunks):
            nc.sync.dma_start(ap[128 * k:128 * (k + 1), :],
                              tile_[:, k, :] if dt is None else tile_[:, k, :].bitcast(dt))

    tap("dbg_x0", X, HC)
    # =============== transformer layers ===============
    for l in range(n_layers):
        X = _layer(nc, tc, d, l, X, dict(
            p_xt=p_xt, p_qk=p_qk, p_v=p_v, p_ctx=p_ctx, p_exp=p_exp,
            p_scr=p_scr, p_gel=p_gel, p_f2a=p_f2a, p_w6=p_w6, p_wi=p_wi, p_wb=p_wb, p_xb=p_xb,
            p_bias=p_bias, p_vec=p_vec, p_vec2=p_vec2, p_lnbc=p_lnbc, p_dram=p_dram, p_sm=p_sm,
            ps_mm=ps_mm, ps_sc=ps_sc, ps_cx=ps_cx, ps_su=ps_su,
            ones_c=ones_c, ones_cb=ones_cb, ones_r=ones_r, ab_t=ab_t, eps_t=eps_t, tap=tap, dd=d,
            ident=ident, ln_pair=ln_pair))

    if DEBUG_TAPS:
        tap("dbg_x1", X, HC)
    # =============== head ===============
    _head(nc, tc, d, X, dict(
        p_qk=p_qk, p_f1=p_f1, p_f2a=p_f2a, p_wi=p_wi, p_lnbc=p_lnbc,
        p_scr=p_scr, p_bias=p_bias, p_sm=p_sm,
        ps_mm=ps_mm, pmat=pmat, out_d=out_d))


def _wfull(nc, pool, tag, src2d, ncols, col0=0, nk=HC):
    """Whole k-major weight tile [128, nk, ncols] (bf16, cast in DMA):
    (p, k, f) <- src2d[128k+p, col0+f] -- contiguous ncols*4B runs."""
    t = pool.tile([128, nk, ncols], BF16, tag=tag)
    src = src2d.rearrange("(k p) f -> p k f", p=128)[:, :, col0:col0 + ncols]
    nc.gpsimd.dma_start(t[:], src)
    return t


def _layer(nc, tc, d, l, X, e):
    p_xt, p_qk, p_v, p_ctx, p_exp = e["p_xt"], e["p_qk"], e["p_v"], e["p_ctx"], e["p_exp"]
    p_scr, p_gel, p_f2a, p_wb = e["p_scr"], e["p_gel"], e["p_f2a"], e["p_wb"]
    p_w6, p_wi, p_xb = e["p_w6"], e["p_wi"], e["p_xb"]
    p_bias, p_vec, p_sm = e["p_bias"], e["p_vec"], e["p_sm"]
    ps_mm, ps_sc, ps_cx, ps_su = e["ps_mm"], e["ps_sc"], e["ps_cx"], e["ps_su"]
    ones_c, ones_cb, ones_r, ab_t = e["ones_c"], e["ones_cb"], e["ones_r"], e["ab_t"]
    dd = e["dd"]
    ident, ln_pair = e["ident"], e["ln_pair"]

    qkvo_b = p_bias.tile([128, 4 * HC], F32, tag="bias")
    for i, bap in enumerate([d["bq"], d["bk"], d["bv"], d["bo"]]):
        nc.sync.dma_start(qkvo_b[:, i * HC:(i + 1) * HC],
                          bap[l].rearrange("(c p) -> p c", p=128))

    # bf16 copy of X for use as the moving operand
    Xb = p_xb.tile([128, HC, T], BF16, tag="xb")
    for k in range(HC):
        nc.vector.tensor_copy(Xb[:, k, :], X[:, k, :].bitcast(F32))

    # ---- Q, K, V projections (transposed layout), V then transposed to token-major
    QT = p_qk.tile([128, HC, T], BF16, tag="qk")
    KT = p_qk.tile([128, HC, T], BF16, tag="qk")
    VT = p_qk.tile([128, HC, T], BF16, tag="qk")
    for w_ap, dst, boff in [(d["Wq"], QT, 0), (d["Wk"], KT, HC),
                            (d["Wv"], VT, 2 * HC)]:
        wt = _wfull(nc, p_w6, "w6", w_ap[l], H)
        for m in range(HC):
            pm_ = ps_mm.tile([128, T], F32, tag="ps_mm")
            for k in range(HC):
                nc.tensor.matmul(pm_[:], wt[:, k, 128 * m:128 * (m + 1)], Xb[:, k, :],
                                 start=(k == 0), stop=(k == HC - 1))
            nc.scalar.activation(dst[:, m, :], pm_[:], AF.Identity,
                                 bias=qkvo_b[:, boff + m:boff + m + 1])

    if l == 0:
        e["tap"]("dbg_q", QT, HC, BF16)
    # V -> token-major [128, TC, H] bf16 via PE transpose
    Vtok = p_v.tile([128, TC, H], BF16, tag="v")
    for c in range(TC):
        for k in range(HC):
            pt = ps_mm.tile([128, 128], BF16, tag="ps_mm")
            nc.tensor.transpose(pt[:], VT[:, k, 128 * c:128 * (c + 1)], ident[:])
            nc.scalar.activation(Vtok[:, c, 128 * k:128 * (k + 1)], pt[:], AF.Copy)

    if l == 0:
        e["tap"]("dbg_v", Vtok, TC, BF16)
    # ---- attention ----
    # ctx accumulated token-major with a fused softmax-sum column, then
    # transposed back to [H, T] layout for the O-projection.
    ctok = p_ctx.tile([128, TC, H], BF16, tag="ctxp")
    for s in range(BPC):
        for h in range(NH):
            kc, po = h // 2, 64 * (h % 2)
            exp_t = [None, None]
            for j in range(2):
                psc = ps_sc.tile([128, 256], F32, tag="ps_sc")
                nc.tensor.matmul(
                    psc[:],
                    KT[po:po + 64, kc, 256 * s + 128 * j:256 * s + 128 * (j + 1)],
                    QT[po:po + 64, kc, 256 * s:256 * (s + 1)],
                    start=True, stop=True)
                et = p_exp.tile([128, 256], BF16, tag="exp")
                nc.scalar.activation(et[:], psc[:], AF.Exp, scale=0.125,
                                     bias=ab_t[:, 2 * s + j:2 * s + j + 1])
                exp_t[j] = et
            if l == 0 and s == 0 and h == 0 and "dbg_exp0" in dd:
                nc.sync.dma_start(dd["dbg_exp0"][:], exp_t[0][:])
                nc.sync.dma_start(dd["dbg_exp1"][:], exp_t[1][:])
            for i in range(2):            # query chunks
                pcx = ps_cx.tile([128, 64], F32, tag="ps_cx")
                psu = ps_su.tile([128, 1], F32, tag="ps_su")
                for j in range(2):
                    lhs = exp_t[j][:, 128 * i:128 * (i + 1)]
                    nc.tensor.matmul(pcx[:], lhs,
                                     Vtok[:, 2 * s + j, 64 * h:64 * h + 64],
                                     start=(j == 0), stop=(j == 1))
                    nc.tensor.matmul(psu[:], lhs, ones_cb[:],
                                     start=(j == 0), stop=(j == 1))
                rec = p_sm.tile([128, 1], F32, tag="rec")
                nc.vector.reciprocal(rec[:], psu[:])
                if l == 0 and s == 0 and h == 0 and i == 0 and "dbg_rec0" in dd:
                    nc.sync.dma_start(dd["dbg_rec0"][:], rec[:])
                nc.vector.tensor_scalar_mul(
                    ctok[:, 2 * s + i, 64 * h:64 * h + 64], pcx[:], rec[:])
    if l == 0:
        e["tap"]("dbg_ctx", ctok, TC, BF16)
    # transpose ctx back to [H, T]
    ctxT = p_qk.tile([128, HC, T], BF16, tag="qk")
    for c in range(TC):
        for k in range(HC):
            pt = ps_mm.tile([128, 128], BF16, tag="ps_mm")
            nc.tensor.transpose(pt[:], ctok[:, c, 128 * k:128 * (k + 1)], ident[:])
            nc.scalar.activation(ctxT[:, k, 128 * c:128 * (c + 1)], pt[:], AF.Copy)

    # ---- O-projection + residual + LN1 ----
    ln1 = ln_pair(d["ln1_s"][l], d["ln1_b"][l], "ln1")
    A = p_xt.tile([128, HC, T], F32R, tag="xt")
    wo_t = _wfull(nc, p_w6, "w6", d["Wo"][l], H)
    for m in range(HC):
        pm_ = ps_mm.tile([128, T], F32, tag="ps_mm")
        for k in range(HC):
            nc.tensor.matmul(pm_[:], wo_t[:, k, 128 * m:128 * (m + 1)], ctxT[:, k, :],
                             start=(k == 0), stop=(k == HC - 1))
        t1 = p_scr.tile([128, T], F32, tag="scr")
        nc.scalar.activation(t1[:], pm_[:], AF.Identity,
                             bias=qkvo_b[:, 3 * HC + m:3 * HC + m + 1])
        nc.vector.tensor_tensor(out=A[:, m, :], in0=t1[:], in1=X[:, m, :].bitcast(F32),
                                op=ALU.add)
    if l == 0:
        e["tap"]("dbg_a", A, HC)
    H2 = _ln_t(nc, A, ln1, e)
    if l == 0:
        e["tap"]("dbg_h2", H2, HC)

    # ---- FFN (quarter passes over FF) ----
    ln2 = ln_pair(d["ln2_s"][l], d["ln2_b"][l], "ln2")
    bi_t = p_bias.tile([128, FFC], F32, tag="bias")
    nc.sync.dma_start(bi_t[:], d["bi"][l].rearrange("(c p) -> p c", p=128))
    bo2_t = p_bias.tile([128, HC], F32, tag="bias")
    nc.sync.dma_start(bo2_t[:], d["bo2"][l].rearrange("(c p) -> p c", p=128))

    H2b = p_xb.tile([128, HC, T], BF16, tag="xb")
    for k in range(HC):
        nc.vector.tensor_copy(H2b[:, k, :], H2[:, k, :].bitcast(F32))

    F2 = p_f2a.tile([128, HC, T], F32, tag="f2a")
    NQ = 4
    QK = FFC // NQ                      # 6 ff-chunks per quarter
    for q in range(NQ):
        wi_t = _wfull(nc, p_wi, "wi", d["Wi"][l], 128 * QK, col0=128 * QK * q)
        gel = p_gel.tile([128, QK, T], BF16, tag="gel")
        for mq in range(QK):
            m = q * QK + mq
            pm_ = ps_mm.tile([128, T], F32, tag="ps_mm")
            for k in range(HC):
                nc.tensor.matmul(pm_[:], wi_t[:, k, 128 * mq:128 * (mq + 1)],
                                 H2b[:, k, :], start=(k == 0), stop=(k == HC - 1))
            nc.scalar.activation(gel[:, mq, :], pm_[:], AF.Gelu,
                                 bias=bi_t[:, m:m + 1])
        wbs = []
        for kq in range(QK):
            m = q * QK + kq
            wb = p_wb.tile([128, 768], BF16, tag="wb")
            nc.gpsimd.dma_start(wb[:], d["Wo2"][l, 128 * m:128 * (m + 1), :])
            wbs.append(wb)
        for o in range(HC):
            pm_ = ps_mm.tile([128, T], F32, tag="ps_mm")
            for kq in range(QK):
                nc.tensor.matmul(pm_[:], wbs[kq][:, 128 * o:128 * (o + 1)],
                                 gel[:, kq, :],
                                 start=(kq == 0), stop=(kq == QK - 1))
            if q == 0:
                nc.scalar.activation(F2[:, o, :], pm_[:], AF.Copy)
            else:
                nc.vector.tensor_tensor(out=F2[:, o, :], in0=F2[:, o, :], in1=pm_[:],
                                        op=ALU.add)
    # residual + bias
    Apre = p_xt.tile([128, HC, T], F32R, tag="xt")
    for o in range(HC):
        t1 = p_scr.tile([128, T], F32, tag="scr")
        nc.vector.tensor_scalar_add(t1[:], F2[:, o, :], bo2_t[:, o:o + 1])
        nc.vector.tensor_tensor(out=Apre[:, o, :], in0=t1[:],
                                in1=H2[:, o, :].bitcast(F32), op=ALU.add)
    if l == 0:
        e["tap"]("dbg_f2", Apre, HC)
    return _ln_t(nc, Apre, ln2, e)


def _ln_t(nc, A, ln_sb, e):
    """LayerNorm over the partition (H) dim for transposed activations.
    A: [128, HC, T] f32r tile. ln_sb: [128, 2*HC] (scale | bias).
    Returns new [128, HC, T] f32r tile."""
    p_xt, p_scr = e["p_xt"], e["p_scr"]
    ps_mm = e["ps_mm"]
    ones_c, ones_r = e["ones_c"], e["ones_r"]

    pmean = e["ps_sc"].tile([1, T], F32, tag="ps_sc")
    for k in range(HC):
        nc.tensor.matmul(pmean[:], ones_c[:], A[:, k, :],
                         start=(k == 0), stop=(k == HC - 1))
    psq = e["ps_sc"].tile([1, T], F32, tag="ps_sc")
    for k in range(HC):
        sq = p_scr.tile([128, T], F32R, tag="scr")
        nc.scalar.activation(sq[:], A[:, k, :].bitcast(F32), AF.Square)
        nc.tensor.matmul(psq[:], ones_c[:], sq[:],
                         start=(k == 0), stop=(k == HC - 1))
    va = e["p_vec"].tile([1, T], F32, tag="vec")   # mu
    vb = e["p_vec"].tile([1, T], F32, tag="vec")   # m2 -> var
    vc = e["p_vec"].tile([1, T], F32, tag="vec")   # musq -> sd -> mu*rstd
    nc.vector.tensor_scalar_mul(va[:], pmean[:], 1.0 / H)
    nc.vector.tensor_scalar_mul(vb[:], psq[:], 1.0 / H)
    nc.vector.tensor_tensor(out=vc[:], in0=va[:], in1=va[:], op=ALU.mult)
    nc.vector.tensor_tensor(out=vb[:], in0=vb[:], in1=vc[:], op=ALU.subtract)
    nc.scalar.activation(vc[:], vb[:], AF.Sqrt, bias=e["eps_t"][0:1, 0:1])
    vec2 = e["p_vec2"].tile([1, 2 * T], F32, tag="vec2")
    rstd, nmr = vec2[:, 0:T], vec2[:, T:2 * T]
    nc.vector.reciprocal(rstd, vc[:])
    nc.vector.tensor_tensor(out=vc[:], in0=va[:], in1=rstd, op=ALU.mult)
    nc.vector.tensor_scalar_mul(nmr, vc[:], -1.0)
    # broadcast rstd and -mu*rstd across partitions via a DRAM bounce
    dscr = e["p_dram"].tile([1, 2 * T], F32, tag="lnscr")
    nc.sync.dma_start(dscr[:], vec2[:])
    bc = e["p_lnbc"].tile([128, 2 * T], F32, tag="lnbc")
    nc.sync.dma_start(bc[:], dscr[:].partition_broadcast(128)[:, 0, :])
    out = p_xt.tile([128, HC, T], F32R, tag="xt")
    for k in range(HC):
        t2 = p_scr.tile([128, T], F32, tag="scr")
        nc.vector.tensor_tensor(out=t2[:], in0=A[:, k, :].bitcast(F32),
                                in1=bc[:, 0:T], op=ALU.mult)
        nc.vector.tensor_tensor(out=t2[:], in0=t2[:], in1=bc[:, T:2 * T], op=ALU.add)
        nc.scalar.activation(out[:, k, :], t2[:], AF.Identity,
                             scale=ln_sb[:, k:k + 1], bias=ln_sb[:, HC + k:HC + k + 1])
    return out


def _head(nc, tc, d, X, e):
    p_qk, p_f1, p_f2a, p_wi, p_lnbc = e["p_qk"], e["p_f1"], e["p_f2a"], e["p_wi"], e["p_lnbc"]
    p_scr, p_bias, p_sm = e["p_scr"], e["p_bias"], e["p_sm"]
    ps_mm = e["ps_mm"]
    pmat, out_d = e["pmat"], e["out_d"]

    # relu(x) transposed, bf16
    reluT = p_qk.tile([128, HC, T], BF16, tag="qk")
    for k in range(HC):
        nc.scalar.activation(reluT[:, k, :], X[:, k, :].bitcast(F32), AF.Relu)
    # f1 = relu(relu(x) @ w1 + b1), transposed layout [M1C, T]
    b1_t = p_bias.tile([128, M1C], F32, tag="bias")
    nc.sync.dma_start(b1_t[:], d["b1"].rearrange("(c p) -> p c", p=128))
    w1_t = p_f2a.tile([128, HC, M1], BF16, tag="f2a")
    nc.gpsimd.dma_start(w1_t[:], d["w1"].rearrange("(k p) f -> p k f", p=128))
    f1 = p_f1.tile([128, M1C, T], BF16, tag="f1")
    for m in range(M1C):
        pm_ = ps_mm.tile([128, T], F32, tag="ps_mm")
        for k in range(HC):
            nc.tensor.matmul(pm_[:], w1_t[:, k, 128 * m:128 * (m + 1)], reluT[:, k, :],
                             start=(k == 0), stop=(k == HC - 1))
        nc.scalar.activation(f1[:, m, :], pm_[:], AF.Relu, bias=b1_t[:, m:m + 1])
    # f2 = f1 @ w2 + b2, token-major [TC, C]
    w2_t = p_wi.tile([128, M1C, C], BF16, tag="wi")
    nc.gpsimd.dma_start(w2_t[:], d["w2"].rearrange("(k p) f -> p k f", p=128))
    b2bc = p_lnbc.tile([128, C], F32, tag="lnbc")
    nc.sync.dma_start(b2bc[:], d["b2"][None, :].partition_broadcast(128)[:, 0, :])
    f2 = p_f2a.tile([128, TC, CPAD], F32R, tag="f2a")
    nc.gpsimd.memset(f2[:].bitcast(F32), 0.0)
    for c in range(TC):
        pm_ = ps_mm.tile([128, C], F32, tag="ps_mm")
        for k in range(M1C):
            nc.tensor.matmul(pm_[:], f1[:, k, 128 * c:128 * (c + 1)], w2_t[:, k, :],
                             start=(k == 0), stop=(k == M1C - 1))
        nc.vector.tensor_tensor(out=f2[:, c, 0:C], in0=pm_[:], in1=b2bc[:],
                                op=ALU.add)

    # pooling + final softmax (N padded to 428 for fp32r)
    CP2 = 428
    for s in range(BPC):
        ppool = ps_mm.tile([128, CP2], F32, tag="ps_mm")
        for j in range(2):
            pm_t = p_sm.tile([128, 128], F32R, tag="pm")
            nc.sync.dma_start(pm_t[:], pmat[256 * s + 128 * j:256 * s + 128 * (j + 1), :].bitcast(F32R))
            nc.tensor.matmul(ppool[:], pm_t[:], f2[:, 2 * s + j, 0:CP2],
                             start=(j == 0), stop=(j == 1))
        for half, src in ((0, ppool[:, 0:C]), (1, f2[:, 2 * s + 1, 0:C].bitcast(F32))):
            ex = p_scr.tile([128, CPAD], F32, tag="scr")
            se = p_sm.tile([128, 2], F32, tag="se")
            nc.scalar.activation(ex[:, 0:C], src, AF.Exp, accum_out=se[:, 0:1])
            nc.vector.reciprocal(se[:, 1:2], se[:, 0:1])
            nc.vector.tensor_scalar_mul(ex[:, 0:C], ex[:, 0:C], se[:, 1:2])
            row0 = 256 * s + 128 * half
            nc.sync.dma_start(out_d[row0:row0 + 128, :], ex[:, 0:C])


# ======================= host side =======================

_PROG_CACHE = {}


def _get_program(n_layers=L):
    if n_layers not in _PROG_CACHE:
        _PROG_CACHE[n_layers] = build_program(n_layers)
    return _PROG_CACHE[n_layers]


def make_in_maps(inputs, n_layers=L):
    """Build per-core input maps from the full-problem inputs dict."""
    f32 = lambda x: np.ascontiguousarray(np.asarray(x), dtype=np.float32)
    enc = np.asarray(inputs["encoded_batch"], dtype=np.int32)
    mask = np.asarray(inputs["mask"], dtype=np.int32)
    wpt = np.asarray(inputs["word_piece_tracked"], dtype=np.int32)

    # pooling matrix P[b, s, w] = 1/cnt[b,w] if seg[b,s]==w else 0
    cum = np.cumsum(wpt, axis=1)                      # [B, W]
    P = np.zeros((B, S, W), dtype=np.float32)
    for b in range(B):
        seg = np.searchsorted(cum[b], np.arange(S), side="right")  # [S]
        valid = seg < W
        P[b, np.arange(S)[valid], seg[valid]] = 1.0 / wpt[b, seg[valid]]

    ab = (1.0 - mask.astype(np.float32)) * -10000.0   # [B, S]

    rep = {}
    for k in ["word_emb", "pos_emb", "type_emb", "emb_ln_s", "emb_ln_b",
              "Wq", "bq", "Wk", "bk", "Wv", "bv", "Wo", "bo", "ln1_s", "ln1_b",
              "Wi", "bi", "Wo2", "bo2", "ln2_s", "ln2_b", "w1", "b1", "w2", "b2"]:
        rep[k] = f32(inputs[k])
    rep["cones"] = np.ones((128, 1), dtype=np.float32)
    rep["ceps"] = np.full((128, 1), EPS, dtype=np.float32)
    rep["crow"] = np.ones((1, 128), dtype=np.float32)

    in_maps = []
    for core in range(N_CORES):
        b0 = core * BPC
        m = dict(rep)
        m["enc"] = enc[b0:b0 + BPC].reshape(T, 1)
        m["ab"] = ab[b0:b0 + BPC].reshape(T)
        m["pmat"] = P[b0:b0 + BPC].reshape(T, W)
        in_maps.append(m)
    return in_maps


def kernel(**inputs):
    nc = _get_program(L)
    in_maps = make_in_maps(inputs, L)
    res = run_bass_kernel_spmd(nc, in_maps, core_ids=list(range(N_CORES)))
    out = np.concatenate([res.results[i]["out"].reshape(BPC, S, C)
                          for i in range(N_CORES)], axis=0)
    return out.astype(np.float32)



# revision 32
# speedup vs baseline: 1.1971x; 1.1971x over previous
"""Trainium2 Bass kernel for the CCG supertagger BERT model.

Data-parallel over batch: 16 samples -> 8 cores x 2 samples.
Key optimizations over the v1 kernel:
  - fp8e4m3 DoubleRow matmuls (0.5 cyc/row) for QKV, V, Wo, Wi, Wo2.
  - LayerNorm scale/bias folded into the next matmul's weights/biases on
    the host; residual re-applications via per-partition scalar ops.
  - LN stats broadcast across partitions by using a [128,128] ones
    stationary (cost only depends on moving rows), rstd computed as
    exp(-0.5*ln(var+eps)) on the scalar engine (stays in the exp table).
  - V projected directly token-major (no PE transposes); softmax sums
    folded into the ctx matmul via a 1/16-column appended to V.
  - All weights pre-cast to fp8/bf16 on the host (4x less HBM traffic).
"""
import numpy as np
import ml_dtypes

import concourse.bass as bass
import concourse.tile as tile
from concourse import bacc, mybir
from concourse.bass_utils import run_bass_kernel_spmd
from concourse.masks import make_identity

F32 = mybir.dt.float32
F32R = mybir.dt.float32r
BF16 = mybir.dt.bfloat16
FP8 = mybir.dt.float8e4
I32 = mybir.dt.int32
AF = mybir.ActivationFunctionType
ALU = mybir.AluOpType
PM = mybir.MatmulPerfMode

B, S, W = 16, 256, 128
V, H, L, NH, DH, FF, C = 30522, 768, 12, 12, 64, 3072, 425
EPS = 1e-12
N_CORES = 8
BPC = B // N_CORES          # samples per core
T = BPC * S                 # tokens per core (512)
HC = H // 128               # 6
FFC = FF // 128             # 24
TC = T // 128               # 4 token chunks
M1 = 1024
M1C = M1 // 128             # 8
CPAD = 448                  # padded class dim for sbuf tiles
SX = 16.0                   # fp8 activation scale
SW = 64.0                   # fp8 weight scale
ISC = 1.0 / (SX * SW)       # psum descale for fp8 x fp8 matmuls
VCOL = 1.0                  # value of the ones-column appended to V


def build_program(n_layers=L):
    nc = bacc.Bacc("TRN2", target_bir_lowering=False, debug=False,
                   num_devices=N_CORES)

    dt_ = lambda name, shape, dt, kind: nc.dram_tensor(name, shape, dt, kind=kind).ap()
    d = {}
    # per-core sharded inputs
    d["enc"] = dt_("enc", [T, 1], I32, "ExternalInput")
    d["ab"] = dt_("ab", [T], F32, "ExternalInput")          # attn bias per key pos
    d["pmat"] = dt_("pmat", [T, 128], F32, "ExternalInput")  # pooling matrices
    # replicated model inputs (host pre-processed)
    d["word_emb"] = dt_("word_emb", [V, H], F32, "ExternalInput")
    d["pos_p"] = dt_("pos_p", [S, H], F32, "ExternalInput")  # pos + type emb
    d["Wq8"] = dt_("Wq8", [L, H, H], FP8, "ExternalInput")
    d["Wk8"] = dt_("Wk8", [L, H, H], FP8, "ExternalInput")
    d["Wv8"] = dt_("Wv8", [L, H, H], FP8, "ExternalInput")
    d["DWv8"] = dt_("DWv8", [L, H, H], FP8, "ExternalInput")
    d["DWi8"] = dt_("DWi8", [L, H, FF], FP8, "ExternalInput")
    d["Wob"] = dt_("Wob", [L, H, H], BF16, "ExternalInput")
    d["Wi8"] = dt_("Wi8", [L, H, FF], FP8, "ExternalInput")
    d["Wo2b"] = dt_("Wo2b", [L, FF, H], BF16, "ExternalInput")
    d["bq_f"] = dt_("bq_f", [L, H], F32, "ExternalInput")
    d["bk_f"] = dt_("bk_f", [L, H], F32, "ExternalInput")
    d["bi_f"] = dt_("bi_f", [L, FF], F32, "ExternalInput")
    d["y2s"] = dt_("y2s", [L, H], F32, "ExternalInput")   # s_in/SX
    d["y2b"] = dt_("y2b", [L, H], F32, "ExternalInput")   # b_in + bo_full
    d["y1s"] = dt_("y1s", [L, H], F32, "ExternalInput")   # ln1_s/SX
    d["y1b"] = dt_("y1b", [L, H], F32, "ExternalInput")   # ln1_b + bo2
    d["hs"] = dt_("hs", [H], F32, "ExternalInput")        # ln2_s[-1]/SX
    d["hb"] = dt_("hb", [H], F32, "ExternalInput")        # ln2_b[-1]
    d["w1b"] = dt_("w1b", [H, M1], BF16, "ExternalInput")
    d["b1"] = dt_("b1", [M1], F32, "ExternalInput")
    d["w2b"] = dt_("w2b", [M1, C], BF16, "ExternalInput")
    d["b2"] = dt_("b2", [C], F32, "ExternalInput")
    d["ones"] = dt_("ones", [128, 128], F32, "ExternalInput")
    d["out_d"] = dt_("out", [T, C], F32, "ExternalOutput")

    with tile.TileContext(nc) as tc:
        from contextlib import ExitStack
        with ExitStack() as ctx:
            _emit(nc, tc, n_layers, d, ctx)
    nc.compile()
    return nc


def _emit(nc, tc, n_layers, d, ctx):
    pool = lambda name, bufs, space="SBUF": ctx.enter_context(
        tc.tile_pool(name=name, bufs=bufs, space=space))

    e = {}
    e["p_x8"] = pool("x8", 3)       # [128, HC, T] fp8 normalized acts (xhat*SX)
    e["p_xt"] = pool("xt", 2)       # [128, HC, T] f32r residual stream
    e["p_qk"] = pool("qk", 2)       # [128, HC, T] bf16 (QT, KT)
    e["p_v"] = pool("v", 1)         # [128, TC, 780] bf16 token-major V + 1/16 cols
    e["p_c8"] = pool("c8", 2)       # [128, TC, H] bf16 ctok / [128, HC, T] fp8 ctxT
    e["p_exp"] = pool("exp", 3)     # [128, 256] bf16 exp tiles
    e["p_big"] = pool("big", 1)     # 12KB arena: emb x0 f32 / gel8 fp8
    e["p_scr"] = pool("scr", 3)     # [128, T] f32 scratch (t2, y2', y1')
    e["p_st"] = pool("st", 3)       # [128, T] f32 LN stats (musq/var/lnv/rstd)
    e["p_sq"] = pool("sq", 2)       # [128, T] f32r square scratch
    e["p_w4"] = pool("w4", 2)       # [128, HC, 2H] fp8 Wq|Wk
    e["p_wv"] = pool("wv", 1)       # [128, HC, 2H] fp8 Wv|DWv
    e["p_wo"] = pool("wo", 1)       # [128, HC, H] bf16 Wo
    e["p_wi"] = pool("wi", 1)       # [128, HC, FF] fp8 (Wi|DWi half)
    e["p_wo2"] = pool("wo2", 1)     # [128, FFC/2, H] bf16 Wo2 half
    e["p_w12"] = pool("w12", 1)     # [128, HC, M1] fp8 w1
    e["p_w2h"] = pool("w2h", 1)     # [128, M1C, C] fp8 w2
    e["p_f1"] = pool("f1", 1)       # [128, M1C, T] fp8
    e["p_f2"] = pool("f2", 1)       # [128, TC, CPAD] f32r
    e["p_pos"] = pool("pos", 1)     # [128, 2, H] bf16 pos embedding
    e["p_bias"] = pool("bias", 6)   # small per-layer bias tiles
    e["p_sm"] = pool("sm", 4)       # small scalars
    e["p_cst"] = pool("cst", 1)     # constants
    e["p_lnbc"] = pool("lnbc", 1)   # [128, C] bf16 b2 broadcast

    e["ps_mm"] = pool("ps_mm", 3, "PSUM")   # [128, 512] f32 main matmul banks
    e["ps_st"] = pool("ps_st", 2, "PSUM")   # [128, 512] f32 LN stat banks
    e["ps_sc"] = pool("ps_sc", 3, "PSUM")   # [128, 256] scores / [128, 65] ctx
    e["ps_cx"] = e["ps_sc"]

    # ---- constants ----
    identb = e["p_cst"].tile([128, 128], BF16, tag="identb")
    make_identity(nc, identb[:])
    ones128 = e["p_cst"].tile([128, 128], F32R, tag="ones128")
    nc.sync.dma_start(ones128[:], d["ones"].bitcast(F32R))
    eps_t = e["p_cst"].tile([128, 1], F32, tag="eps")
    nc.gpsimd.memset(eps_t[:], EPS)
    e["identb"] = identb
    e["ones128"], e["eps_t"] = ones128, eps_t

    ab_t = e["p_cst"].tile([128, TC], F32, tag="ab")
    nc.sync.dma_start(ab_t[:], d["ab"].rearrange("(c p) -> p c", p=128))
    e["ab_t"] = ab_t

    def colvec(ap_1d, n, tag, pool="p_bias"):
        t = e[pool].tile([128, n], F32, tag="bias")
        nc.sync.dma_start(t[:], ap_1d.rearrange("(c p) -> p c", p=128))
        return t
    e["colvec"] = colvec

    # =============== embedding ===============
    x0 = e["p_big"].tile([128, TC, H], F32, tag="big")
    for c in range(TC):
        idx_t = e["p_sm"].tile([128, 1], I32, tag="sm")
        nc.sync.dma_start(idx_t[:], d["enc"][128 * c:128 * (c + 1), :])
        nc.gpsimd.indirect_dma_start(
            out=x0[:, c, :], out_offset=None, in_=d["word_emb"][:],
            in_offset=bass.IndirectOffsetOnAxis(ap=idx_t[:, :1], axis=0))
    pos_t = e["p_pos"].tile([128, 2, H], BF16, tag="pos")
    nc.gpsimd.dma_start(pos_t[:, 0, :], d["pos_p"][0:128, :])
    nc.gpsimd.dma_start(pos_t[:, 1, :], d["pos_p"][128:256, :])

    xq8 = e["p_x8"].tile([128, HC, T], FP8, tag="x8")
    xh = e["p_xt"].tile([128, HC, T], F32R, tag="xt")
    for c in range(TC):
        xc = x0[:, c, :]
        nc.vector.tensor_tensor(out=xc, in0=xc, in1=pos_t[:, c % 2, :], op=ALU.add)
        # token-major layernorm (per-partition stats)
        su = e["p_sm"].tile([128, 4], F32, tag="sm")
        nc.vector.reduce_sum(out=su[:, 0:1], in_=xc, axis=mybir.AxisListType.X)
        sq = e["p_scr"].tile([128, H], F32, tag="scr")
        nc.scalar.activation(sq[:], xc, AF.Square, accum_out=su[:, 1:2])
        st = e["p_sm"].tile([128, 4], F32, tag="sm")
        nc.vector.tensor_scalar_mul(st[:, 0:1], su[:, 0:1], 1.0 / H)      # mu
        nc.vector.tensor_scalar_mul(st[:, 1:2], su[:, 1:2], 1.0 / H)      # m2
        nc.vector.tensor_tensor(out=st[:, 2:3], in0=st[:, 0:1], in1=st[:, 0:1], op=ALU.mult)
        nc.vector.tensor_tensor(out=st[:, 3:4], in0=st[:, 1:2], in1=st[:, 2:3], op=ALU.subtract)
        sd = e["p_sm"].tile([128, 2], F32, tag="sm")
        nc.scalar.activation(sd[:, 0:1], st[:, 3:4], AF.Ln, bias=eps_t[:, 0:1])
        nc.scalar.activation(sd[:, 1:2], sd[:, 0:1], AF.Exp, scale=-0.5)
        xcr = e["p_scr"].tile([128, H], BF16, tag="scr")
        nc.vector.tensor_scalar(out=xcr[:], in0=xc, scalar1=st[:, 0:1], scalar2=sd[:, 1:2],
                                op0=ALU.subtract, op1=ALU.mult)
        for k in range(HC):
            pt = e["ps_mm"].tile([128, 128], BF16, tag="ps_mm")
            nc.tensor.transpose(pt[:], xcr[:, 128 * k:128 * (k + 1)], identb[:])
            nc.scalar.activation(xh[:, k, 128 * c:128 * (c + 1)], pt[:], AF.Copy)
            nc.vector.tensor_scalar_mul(xq8[:, k, 128 * c:128 * (c + 1)],
                                        xh[:, k, 128 * c:128 * (c + 1)].bitcast(F32),
                                        SX)

    # =============== transformer layers ===============
    for l in range(n_layers):
        xq8, xh = _layer(nc, d, l, xq8, xh, e)

    # =============== head ===============
    _head(nc, d, xh, e)


def _wload(nc, dst_ap, src2d):
    """DMA a [K, F] dram weight into a k-major [128, K/128, F] SBUF AP."""
    nc.gpsimd.dma_start(dst_ap, src2d.rearrange("(k p) f -> p k f", p=128))


def _ln_finish(nc, e, A, mean_ps, sq_ps):
    """Partition-dim LN from broadcast stat psums; returns fp8 xhat*SX tile."""
    musq = e["p_st"].tile([128, T], F32, tag="st")
    nc.scalar.activation(musq[:], mean_ps[:], AF.Square, scale=1.0 / H)
    var = e["p_st"].tile([128, T], F32, tag="st")
    nc.vector.scalar_tensor_tensor(out=var[:], in0=sq_ps[:], scalar=1.0 / H,
                                   in1=musq[:], op0=ALU.mult, op1=ALU.subtract)
    lnv = e["p_st"].tile([128, T], F32, tag="st")
    nc.scalar.activation(lnv[:], var[:], AF.Ln, bias=e["eps_t"][:, 0:1])
    rstd = e["p_st"].tile([128, T], F32, tag="st")
    nc.scalar.activation(rstd[:], lnv[:], AF.Exp, scale=-0.5)
    xq8n = e["p_x8"].tile([128, HC, T], FP8, tag="x8")
    for k in range(HC):
        t2 = e["p_scr"].tile([128, T], F32, tag="scr")
        nc.vector.scalar_tensor_tensor(out=t2[:], in0=mean_ps[:], scalar=-1.0 / H,
                                       in1=A[:, k, :].bitcast(F32),
                                       op0=ALU.mult, op1=ALU.add)
        # xhat overwrites the residual tile in place (read by the y-ops)
        nc.vector.tensor_tensor(out=A[:, k, :], in0=t2[:], in1=rstd[:], op=ALU.mult)
        nc.vector.tensor_scalar_mul(xq8n[:, k, :], A[:, k, :].bitcast(F32), SX)
    return xq8n


def _layer(nc, d, l, xq8, xh_in, e):
    ps_mm, ps_st, ps_sc, ps_cx = e["ps_mm"], e["ps_st"], e["ps_sc"], e["ps_cx"]
    ones128, ab_t = e["ones128"], e["ab_t"]
    colvec = e["colvec"]

    # ---- per-layer weights & biases ----
    wqk = e["p_w4"].tile([128, HC, 2 * H], FP8, tag="w4")
    for i, wn in enumerate(["Wq8", "Wk8"]):
        _wload(nc, wqk[:, :, H * i:H * (i + 1)], d[wn][l])
    wv = e["p_wv"].tile([128, HC, 2 * H], FP8, tag="wv")
    _wload(nc, wv[:, :, 0:H], d["Wv8"][l])
    _wload(nc, wv[:, :, H:2 * H], d["DWv8"][l])
    wo = e["p_wo"].tile([128, HC, H], BF16, tag="wo")
    _wload(nc, wo[:], d["Wob"][l])
    bqk = e["p_bias"].tile([128, 2 * HC], F32, tag="bias")
    nc.sync.dma_start(bqk[:, 0:HC], d["bq_f"][l].rearrange("(c p) -> p c", p=128))
    nc.sync.dma_start(bqk[:, HC:2 * HC], d["bk_f"][l].rearrange("(c p) -> p c", p=128))
    yv = e["p_bias"].tile([128, 4 * HC], F32, tag="bias")
    for i, yn in enumerate(["y2s", "y2b", "y1s", "y1b"]):
        nc.sync.dma_start(yv[:, HC * i:HC * (i + 1)],
                          d[yn][l].rearrange("(c p) -> p c", p=128))
    bi_t = colvec(d["bi_f"][l], FFC, "bi")

    # ---- Q, K projections (dh-major) ----
    QT = e["p_qk"].tile([128, HC, T], BF16, tag="qk")
    KT = e["p_qk"].tile([128, HC, T], BF16, tag="qk")
    for pi, dst in enumerate([QT, KT]):
        for m in range(HC):
            pm_ = ps_mm.tile([128, T], F32, tag="ps_mm")
            for j in range(HC // 2):
                nc.tensor.matmul(
                    pm_[:], wqk[:, 2 * j:2 * j + 2, H * pi + 128 * m:H * pi + 128 * (m + 1)],
                    xq8[:, 2 * j:2 * j + 2, :],
                    start=(j == 0), stop=(j == HC // 2 - 1), perf_mode=PM.DoubleRow)
            nc.scalar.activation(dst[:, m, :], pm_[:], AF.Identity, scale=ISC,
                                 bias=bqk[:, HC * pi + m:HC * pi + m + 1])

    # ---- V projection, token-major, with 1/SX columns for softmax sums ----
    Vt = e["p_v"].tile([128, TC, 65 * NH], BF16, tag="v")
    vones = Vt[:].rearrange("p c (h x) -> p c h x", h=NH)[:, :, :, 64:65]
    nc.gpsimd.memset(vones, VCOL)
    for c in range(TC):
        for hf in range(2):
            pm_ = ps_mm.tile([128, 384], F32, tag="ps_mm")
            for dw in range(2):
                for j in range(HC // 2):
                    nc.tensor.matmul(
                        pm_[:], xq8[:, 2 * j:2 * j + 2, 128 * c:128 * (c + 1)],
                        wv[:, 2 * j:2 * j + 2, H * dw + 384 * hf:H * dw + 384 * (hf + 1)],
                        start=(dw == 0 and j == 0),
                        stop=(dw == 1 and j == HC // 2 - 1), perf_mode=PM.DoubleRow)
            dstv = Vt[:, c, 390 * hf:390 * (hf + 1)].rearrange(
                "p (h x) -> p h x", h=6)[:, :, 0:64]
            nc.scalar.activation(dstv, pm_[:].rearrange("p (h x) -> p h x", h=6),
                                 AF.Copy, scale=ISC)

    # ---- attention ----
    ctok = e["p_c8"].tile([128, TC, H], BF16, tag="c8")
    for s in range(BPC):
        for h in range(NH):
            kc, po = h // 2, 64 * (h % 2)
            exp_t = [None, None]
            for j in range(2):
                psc = ps_sc.tile([128, 256], F32, tag="ps_sc")
                nc.tensor.matmul(
                    psc[:],
                    KT[po:po + 64, kc, 256 * s + 128 * j:256 * s + 128 * (j + 1)],
                    QT[po:po + 64, kc, 256 * s:256 * (s + 1)],
                    start=True, stop=True)
                et = e["p_exp"].tile([128, 256], BF16, tag="exp")
                nc.scalar.activation(et[:], psc[:], AF.Exp, scale=0.125,
                                     bias=ab_t[:, 2 * s + j:2 * s + j + 1])
                exp_t[j] = et
            for i in range(2):
                pcx = ps_cx.tile([128, 65], F32, tag="ps_sc")
                for j in range(2):
                    nc.tensor.matmul(pcx[:], exp_t[j][:, 128 * i:128 * (i + 1)],
                                     Vt[:, 2 * s + j, 65 * h:65 * h + 65],
                                     start=(j == 0), stop=(j == 1))
                rec = e["p_sm"].tile([128, 1], F32, tag="sm")
                nc.vector.reciprocal(rec[:], pcx[:, 64:65])
                nc.vector.tensor_scalar_mul(
                    ctok[:, 2 * s + i, 64 * h:64 * h + 64], pcx[:, 0:64], rec[:])

    # ---- transpose ctx to [H, T] bf16 ----
    ctxT = e["p_c8"].tile([128, HC, T], BF16, tag="c8")
    for c in range(TC):
        for k in range(HC):
            pt = ps_mm.tile([128, 128], BF16, tag="ps_mm")
            nc.tensor.transpose(pt[:], ctok[:, c, 128 * k:128 * (k + 1)],
                                e["identb"][:])
            nc.scalar.activation(ctxT[:, k, 128 * c:128 * (c + 1)], pt[:], AF.Copy)

    # ---- O projection + residual + LN1 stats ----
    A = e["p_xt"].tile([128, HC, T], F32R, tag="xt")
    mean_ps = ps_st.tile([128, T], F32, tag="ps_st")
    sq_ps = ps_st.tile([128, T], F32, tag="ps_st")
    for o in range(HC):
        pm_ = ps_mm.tile([128, T], F32, tag="ps_mm")
        for k in range(HC):
            nc.tensor.matmul(pm_[:], wo[:, k, 128 * o:128 * (o + 1)],
                             ctxT[:, k, :], start=(k == 0), stop=(k == HC - 1))
        y2o = e["p_scr"].tile([128, T], F32, tag="scr")
        nc.scalar.activation(y2o[:], xh_in[:, o, :].bitcast(F32), AF.Identity,
                             scale=yv[:, o:o + 1], bias=yv[:, HC + o:HC + o + 1])
        nc.vector.scalar_tensor_tensor(out=A[:, o, :], in0=pm_[:], scalar=1.0,
                                       in1=y2o[:], op0=ALU.mult, op1=ALU.add)
        nc.tensor.matmul(mean_ps[:], ones128[:], A[:, o, :],
                         start=(o == 0), stop=(o == HC - 1))
        sq = e["p_sq"].tile([128, T], F32R, tag="sq")
        nc.scalar.activation(sq[:], A[:, o, :].bitcast(F32), AF.Square)
        nc.tensor.matmul(sq_ps[:], ones128[:], sq[:],
                         start=(o == 0), stop=(o == HC - 1))
    xq81 = _ln_finish(nc, e, A, mean_ps, sq_ps)


    # ---- FFN: two FF-chunk halves; Wi fp8+delta compensated, Wo2 bf16 ----
    Apre = e["p_xt"].tile([128, HC, T], F32R, tag="xt")
    mean2 = ps_st.tile([128, T], F32, tag="ps_st")
    sq2 = ps_st.tile([128, T], F32, tag="ps_st")
    FH = FF // 2                       # 1536 ff cols per half
    KH = FFC // 2                      # 12 ff chunks per half
    for wh in range(2):
        wi = e["p_wi"].tile([128, HC, FF], FP8, tag="wi")
        nc.gpsimd.dma_start(
            wi[:, :, 0:FH], d["Wi8"][l].rearrange("(k p) f -> p k f", p=128)
            [:, :, FH * wh:FH * (wh + 1)])
        nc.gpsimd.dma_start(
            wi[:, :, FH:FF], d["DWi8"][l].rearrange("(k p) f -> p k f", p=128)
            [:, :, FH * wh:FH * (wh + 1)])
        wo2 = e["p_wo2"].tile([128, KH, H], BF16, tag="wo2")
        _wload(nc, wo2[:], d["Wo2b"][l, FH * wh:FH * (wh + 1)])
        gel = e["p_big"].tile([128, KH, T], BF16, tag="big")
        for mi in range(KH):
            m = KH * wh + mi
            pm_ = ps_mm.tile([128, T], F32, tag="ps_mm")
            for dw in range(2):
                for j in range(HC // 2):
                    nc.tensor.matmul(
                        pm_[:], wi[:, 2 * j:2 * j + 2,
                                   FH * dw + 128 * mi:FH * dw + 128 * (mi + 1)],
                        xq81[:, 2 * j:2 * j + 2, :],
                        start=(dw == 0 and j == 0),
                        stop=(dw == 1 and j == HC // 2 - 1), perf_mode=PM.DoubleRow)
            nc.scalar.activation(gel[:, mi, :], pm_[:], AF.Gelu, scale=ISC,
                                 bias=bi_t[:, m:m + 1])
        for o in range(HC):
            pm_ = ps_mm.tile([128, T], F32, tag="ps_mm")
            for k in range(KH):
                nc.tensor.matmul(pm_[:], wo2[:, k, 128 * o:128 * (o + 1)],
                                 gel[:, k, :], start=(k == 0), stop=(k == KH - 1))
            if wh == 0:
                y1o = e["p_scr"].tile([128, T], F32, tag="scr")
                nc.scalar.activation(y1o[:], A[:, o, :].bitcast(F32), AF.Identity,
                                     scale=yv[:, 2 * HC + o:2 * HC + o + 1],
                                     bias=yv[:, 3 * HC + o:3 * HC + o + 1])
                nc.vector.scalar_tensor_tensor(out=Apre[:, o, :], in0=pm_[:],
                                               scalar=1.0, in1=y1o[:],
                                               op0=ALU.mult, op1=ALU.add)
            else:
                nc.vector.scalar_tensor_tensor(out=Apre[:, o, :], in0=pm_[:],
                                               scalar=1.0,
                                               in1=Apre[:, o, :].bitcast(F32),
                                               op0=ALU.mult, op1=ALU.add)
                nc.tensor.matmul(mean2[:], ones128[:], Apre[:, o, :],
                                 start=(o == 0), stop=(o == HC - 1))
                sq = e["p_sq"].tile([128, T], F32R, tag="sq")
                nc.scalar.activation(sq[:], Apre[:, o, :].bitcast(F32), AF.Square)
                nc.tensor.matmul(sq2[:], ones128[:], sq[:],
                                 start=(o == 0), stop=(o == HC - 1))
    return _ln_finish(nc, e, Apre, mean2, sq2), Apre


def _head(nc, d, xh, e):
    ps_mm = e["ps_mm"]
    colvec = e["colvec"]
    hs_t = colvec(d["hs"], HC, "hs")
    hb_t = colvec(d["hb"], HC, "hb")
    # relu(x) with final LN scale/bias fused, bf16
    reluT = e["p_qk"].tile([128, HC, T], BF16, tag="qk")
    for k in range(HC):
        nc.scalar.activation(reluT[:, k, :], xh[:, k, :].bitcast(F32), AF.Relu,
                             scale=hs_t[:, k:k + 1], bias=hb_t[:, k:k + 1])
    # f1 = relu(relu(x) @ w1 + b1), [M1C, T] bf16
    b1_t = colvec(d["b1"], M1C, "b1")
    w1t = e["p_w12"].tile([128, HC, M1], BF16, tag="w12")
    _wload(nc, w1t[:], d["w1b"])
    f1 = e["p_f1"].tile([128, M1C, T], BF16, tag="f1")
    for m in range(M1C):
        pm_ = ps_mm.tile([128, T], F32, tag="ps_mm")
        for k in range(HC):
            nc.tensor.matmul(pm_[:], w1t[:, k, 128 * m:128 * (m + 1)], reluT[:, k, :],
                             start=(k == 0), stop=(k == HC - 1))
        nc.scalar.activation(f1[:, m, :], pm_[:], AF.Relu, bias=b1_t[:, m:m + 1])
    # f2 = f1 @ w2 + b2, token-major [TC, C]
    w2t = e["p_w2h"].tile([128, M1C, C], BF16, tag="w2h")
    _wload(nc, w2t[:], d["w2b"])
    b2bc = e["p_lnbc"].tile([128, C], BF16, tag="lnbc")
    nc.gpsimd.dma_start(b2bc[:], d["b2"][None, :].partition_broadcast(128)[:, 0, :])
    f2 = e["p_f2"].tile([128, TC, CPAD], F32R, tag="f2")
    nc.gpsimd.memset(f2[:].bitcast(F32), 0.0)
    for c in range(TC):
        pm_ = ps_mm.tile([128, C], F32, tag="ps_mm")
        for k in range(M1C):
            nc.tensor.matmul(pm_[:], f1[:, k, 128 * c:128 * (c + 1)], w2t[:, k, :],
                             start=(k == 0), stop=(k == M1C - 1))
        nc.vector.tensor_tensor(out=f2[:, c, 0:C], in0=pm_[:], in1=b2bc[:],
                                op=ALU.add)
    # pooling + final softmax (N padded to 428 for fp32r)
    CP2 = 428
    for s in range(BPC):
        ppool = ps_mm.tile([128, CP2], F32, tag="ps_mm")
        for j in range(2):
            pm_t = e["p_exp"].tile([128, 128], F32R, tag="exp")
            nc.sync.dma_start(pm_t[:], d["pmat"][256 * s + 128 * j:256 * s + 128 * (j + 1), :].bitcast(F32R))
            nc.tensor.matmul(ppool[:], pm_t[:], f2[:, 2 * s + j, 0:CP2],
                             start=(j == 0), stop=(j == 1))
        for half, src in ((0, ppool[:, 0:C]), (1, f2[:, 2 * s + 1, 0:C].bitcast(F32))):
            ex = e["p_scr"].tile([128, CPAD], F32, tag="scr")
            se = e["p_sm"].tile([128, 2], F32, tag="sm")
            nc.scalar.activation(ex[:, 0:C], src, AF.Exp, accum_out=se[:, 0:1])
            nc.vector.reciprocal(se[:, 1:2], se[:, 0:1])
            nc.vector.tensor_scalar_mul(ex[:, 0:C], ex[:, 0:C], se[:, 1:2])
            row0 = 256 * s + 128 * half
            nc.sync.dma_start(d["out_d"][row0:row0 + 128, :], ex[:, 0:C])


# ======================= host side =======================

_PROG_CACHE = {}


def _get_program(n_layers=L):
    if n_layers not in _PROG_CACHE:
        _PROG_CACHE[n_layers] = build_program(n_layers)
    return _PROG_CACHE[n_layers]


def make_in_maps(inputs, n_layers=L):
    """Build per-core input maps; fold LN scale/bias into weights, pre-cast
    weights to fp8/bf16."""
    f32 = lambda x: np.ascontiguousarray(np.asarray(x), dtype=np.float32)
    fp8 = lambda x: np.ascontiguousarray(
        np.asarray(x, dtype=np.float32).astype(ml_dtypes.float8_e4m3fn))
    bf16 = lambda x: np.ascontiguousarray(
        np.asarray(x, dtype=np.float32).astype(ml_dtypes.bfloat16))
    enc = np.asarray(inputs["encoded_batch"], dtype=np.int32)
    mask = np.asarray(inputs["mask"], dtype=np.int32)
    wpt = np.asarray(inputs["word_piece_tracked"], dtype=np.int32)

    Wq, Wk, Wv, Wo = (f32(inputs[k]) for k in ["Wq", "Wk", "Wv", "Wo"])
    Wi, Wo2 = f32(inputs["Wi"]), f32(inputs["Wo2"])
    bq, bk, bv, bo = (f32(inputs[k]) for k in ["bq", "bk", "bv", "bo"])
    bi, bo2 = f32(inputs["bi"]), f32(inputs["bo2"])
    ln1_s, ln1_b = f32(inputs["ln1_s"]), f32(inputs["ln1_b"])
    ln2_s, ln2_b = f32(inputs["ln2_s"]), f32(inputs["ln2_b"])

    s_in = np.empty((L, H), np.float32)
    b_in = np.empty((L, H), np.float32)
    s_in[0], b_in[0] = f32(inputs["emb_ln_s"]), f32(inputs["emb_ln_b"])
    s_in[1:], b_in[1:] = ln2_s[:L - 1], ln2_b[:L - 1]

    Wq8 = np.empty((L, H, H), ml_dtypes.float8_e4m3fn)
    Wk8 = np.empty_like(Wq8)
    Wv8 = np.empty_like(Wq8)
    Wi8 = np.empty((L, H, FF), ml_dtypes.float8_e4m3fn)
    bq_f = np.empty((L, H), np.float32)
    bk_f = np.empty_like(bq_f)
    y2b = np.empty_like(bq_f)
    bi_f = np.empty((L, FF), np.float32)
    DWv8 = np.empty_like(Wq8)
    DWi8 = np.empty_like(Wi8)
    for l in range(L):
        Wq8[l] = fp8(s_in[l][:, None] * Wq[l] * SW)
        Wk8[l] = fp8(s_in[l][:, None] * Wk[l] * SW)
        wv_t = s_in[l][:, None] * Wv[l] * SW
        Wv8[l] = fp8(wv_t)
        DWv8[l] = fp8(wv_t - Wv8[l].astype(np.float32))
        wi_t = ln1_s[l][:, None] * Wi[l] * SW
        Wi8[l] = fp8(wi_t)
        DWi8[l] = fp8(wi_t - Wi8[l].astype(np.float32))
        bq_f[l] = b_in[l] @ Wq[l] + bq[l]
        bk_f[l] = b_in[l] @ Wk[l] + bk[l]
        bv_full = b_in[l] @ Wv[l] + bv[l]
        bo_full = bv_full @ Wo[l] + bo[l]
        y2b[l] = b_in[l] + bo_full
        bi_f[l] = b_in_ff = ln1_b[l] @ Wi[l] + bi[l]

    # pooling matrix P[b, s, w] = 1/cnt[b,w] if seg[b,s]==w else 0
    cum = np.cumsum(wpt, axis=1)
    P = np.zeros((B, S, W), dtype=np.float32)
    for b in range(B):
        seg = np.searchsorted(cum[b], np.arange(S), side="right")
        valid = seg < W
        P[b, np.arange(S)[valid], seg[valid]] = 1.0 / wpt[b, seg[valid]]

    ab = (1.0 - mask.astype(np.float32)) * -10000.0

    rep = dict(
        word_emb=f32(inputs["word_emb"]),
        pos_p=f32(inputs["pos_emb"]) + f32(inputs["type_emb"])[0][None, :],
        Wq8=Wq8, Wk8=Wk8, Wv8=Wv8, DWv8=DWv8, Wob=bf16(Wo),
        Wi8=Wi8, DWi8=DWi8, Wo2b=bf16(Wo2),
        bq_f=bq_f, bk_f=bk_f, bi_f=bi_f,
        y2s=s_in, y2b=y2b,
        y1s=ln1_s, y1b=ln1_b + bo2,
        hs=ln2_s[n_layers - 1], hb=ln2_b[n_layers - 1],
        w1b=bf16(inputs["w1"]), b1=f32(inputs["b1"]),
        w2b=bf16(inputs["w2"]), b2=f32(inputs["b2"]),
        ones=np.ones((128, 128), np.float32),
    )

    in_maps = []
    for core in range(N_CORES):
        b0 = core * BPC
        m = dict(rep)
        m["enc"] = enc[b0:b0 + BPC].reshape(T, 1)
        m["ab"] = ab[b0:b0 + BPC].reshape(T)
        m["pmat"] = P[b0:b0 + BPC].reshape(T, W)
        in_maps.append(m)
    return in_maps


def kernel(**inputs):
    nc = _get_program(L)
    in_maps = make_in_maps(inputs, L)
    res = run_bass_kernel_spmd(nc, in_maps, core_ids=list(range(N_CORES)))
    out = np.concatenate([res.results[i]["out"].reshape(BPC, S, C)
                          for i in range(N_CORES)], axis=0)
    return out.astype(np.float32)


# revision 33
# speedup vs baseline: 1.2224x; 1.0211x over previous
"""Trainium2 Bass kernel for the CCG supertagger BERT model.

Data-parallel over batch: 16 samples -> 8 cores x 2 samples.
Key optimizations over the v1 kernel:
  - fp8e4m3 DoubleRow matmuls (0.5 cyc/row) for QKV, V, Wo, Wi, Wo2.
  - LayerNorm scale/bias folded into the next matmul's weights/biases on
    the host; residual re-applications via per-partition scalar ops.
  - LN stats broadcast across partitions by using a [128,128] ones
    stationary (cost only depends on moving rows), rstd computed as
    exp(-0.5*ln(var+eps)) on the scalar engine (stays in the exp table).
  - V projected directly token-major (no PE transposes); softmax sums
    folded into the ctx matmul via a 1/16-column appended to V.
  - All weights pre-cast to fp8/bf16 on the host (4x less HBM traffic).
"""
import numpy as np
import ml_dtypes

import concourse.bass as bass
import concourse.tile as tile
from concourse import bacc, mybir
from concourse.bass_utils import run_bass_kernel_spmd
from concourse.masks import make_identity

F32 = mybir.dt.float32
F32R = mybir.dt.float32r
BF16 = mybir.dt.bfloat16
FP8 = mybir.dt.float8e4
I32 = mybir.dt.int32
AF = mybir.ActivationFunctionType
ALU = mybir.AluOpType
PM = mybir.MatmulPerfMode

B, S, W = 16, 256, 128
V, H, L, NH, DH, FF, C = 30522, 768, 12, 12, 64, 3072, 425
EPS = 1e-12
N_CORES = 8
BPC = B // N_CORES          # samples per core
T = BPC * S                 # tokens per core (512)
HC = H // 128               # 6
FFC = FF // 128             # 24
TC = T // 128               # 4 token chunks
M1 = 1024
M1C = M1 // 128             # 8
CPAD = 448                  # padded class dim for sbuf tiles
SX = 16.0                   # fp8 activation scale
SW = 64.0                   # fp8 weight scale
ISC = 1.0 / (SX * SW)       # psum descale for fp8 x fp8 matmuls
VCOL = 1.0                  # value of the ones-column appended to V


def build_program(n_layers=L):
    nc = bacc.Bacc("TRN2", target_bir_lowering=False, debug=False,
                   num_devices=N_CORES)

    dt_ = lambda name, shape, dt, kind: nc.dram_tensor(name, shape, dt, kind=kind).ap()
    d = {}
    # per-core sharded inputs
    d["enc"] = dt_("enc", [T, 1], I32, "ExternalInput")
    d["ab"] = dt_("ab", [T], F32, "ExternalInput")          # attn bias per key pos
    d["pmat"] = dt_("pmat", [T, 128], F32, "ExternalInput")  # pooling matrices
    # replicated model inputs (host pre-processed)
    d["word_emb"] = dt_("word_emb", [V, H], F32, "ExternalInput")
    d["pos_p"] = dt_("pos_p", [S, H], F32, "ExternalInput")  # pos + type emb
    d["Wq8"] = dt_("Wq8", [L, H, H], FP8, "ExternalInput")
    d["Wk8"] = dt_("Wk8", [L, H, H], FP8, "ExternalInput")
    d["Wv8"] = dt_("Wv8", [L, H, H], FP8, "ExternalInput")
    d["DWv8"] = dt_("DWv8", [L, H, H], FP8, "ExternalInput")
    d["DWi8"] = dt_("DWi8", [L, H, FF], FP8, "ExternalInput")
    d["Wob"] = dt_("Wob", [L, H, H], BF16, "ExternalInput")
    d["Wi8"] = dt_("Wi8", [L, H, FF], FP8, "ExternalInput")
    d["Wo2b"] = dt_("Wo2b", [L, FF, H], BF16, "ExternalInput")
    d["bq_f"] = dt_("bq_f", [L, H], F32, "ExternalInput")
    d["bk_f"] = dt_("bk_f", [L, H], F32, "ExternalInput")
    d["bi_f"] = dt_("bi_f", [L, FF], F32, "ExternalInput")
    d["y2s"] = dt_("y2s", [L, H], F32, "ExternalInput")   # s_in/SX
    d["y2b"] = dt_("y2b", [L, H], F32, "ExternalInput")   # b_in + bo_full
    d["y1s"] = dt_("y1s", [L, H], F32, "ExternalInput")   # ln1_s/SX
    d["y1b"] = dt_("y1b", [L, H], F32, "ExternalInput")   # ln1_b + bo2
    d["hs"] = dt_("hs", [H], F32, "ExternalInput")        # ln2_s[-1]/SX
    d["hb"] = dt_("hb", [H], F32, "ExternalInput")        # ln2_b[-1]
    d["w1b"] = dt_("w1b", [H, M1], BF16, "ExternalInput")
    d["b1"] = dt_("b1", [M1], F32, "ExternalInput")
    d["w2b"] = dt_("w2b", [M1, C], BF16, "ExternalInput")
    d["b2"] = dt_("b2", [C], F32, "ExternalInput")
    d["ones"] = dt_("ones", [128, 128], F32, "ExternalInput")
    d["out_d"] = dt_("out", [T, C], F32, "ExternalOutput")

    with tile.TileContext(nc) as tc:
        from contextlib import ExitStack
        with ExitStack() as ctx:
            _emit(nc, tc, n_layers, d, ctx)
    nc.compile()
    return nc


def _emit(nc, tc, n_layers, d, ctx):
    pool = lambda name, bufs, space="SBUF": ctx.enter_context(
        tc.tile_pool(name=name, bufs=bufs, space=space))

    e = {}
    e["p_x8"] = pool("x8", 3)       # [128, HC, T] fp8 normalized acts (xhat*SX)
    e["p_xt"] = pool("xt", 2)       # [128, HC, T] f32r residual stream
    e["p_qk"] = pool("qk", 2)       # [128, HC, T] bf16 (QT, KT)
    e["p_v"] = pool("v", 1)         # [128, TC, 780] fp8 token-major V + ones cols
    e["p_c8"] = pool("c8", 2)       # [128, TC, H] bf16 ctok / [128, HC, T] fp8 ctxT
    e["p_exp"] = pool("exp", 3)     # [128, 256] bf16 exp tiles
    e["p_big"] = pool("big", 1)     # 12KB arena: emb x0 f32 / gel8 fp8
    e["p_scr"] = pool("scr", 3)     # [128, T] f32 scratch (t2, y2', y1')
    e["p_st"] = pool("st", 3)       # [128, T] f32 LN stats (musq/var/lnv/rstd)
    e["p_sq"] = pool("sq", 2)       # [128, T] f32r square scratch
    e["p_w4"] = pool("w4", 2)       # [128, HC, 2H] fp8 Wq|Wk
    e["p_wv"] = pool("wv", 1)       # [128, HC, 2H] fp8 Wv|DWv
    e["p_wo"] = pool("wo", 1)       # [128, HC, H] bf16 Wo
    e["p_wi"] = pool("wi", 1)       # [128, HC, FF] fp8 (Wi|DWi half)
    e["p_wo2"] = pool("wo2", 1)     # [128, FFC/2, H] bf16 Wo2 half
    e["p_w12"] = pool("w12", 1)     # [128, HC, M1] fp8 w1
    e["p_w2h"] = pool("w2h", 1)     # [128, M1C, C] fp8 w2
    e["p_f1"] = pool("f1", 1)       # [128, M1C, T] fp8
    e["p_f2"] = pool("f2", 1)       # [128, TC, CPAD] f32r
    e["p_pos"] = pool("pos", 1)     # [128, 2, H] bf16 pos embedding
    e["p_bias"] = pool("bias", 6)   # small per-layer bias tiles
    e["p_sm"] = pool("sm", 4)       # small scalars
    e["p_cst"] = pool("cst", 1)     # constants
    e["p_lnbc"] = pool("lnbc", 1)   # [128, C] bf16 b2 broadcast

    e["ps_mm"] = pool("ps_mm", 4, "PSUM")   # [128, 512] f32 main matmul banks
    e["ps_sc"] = pool("ps_sc", 4, "PSUM")   # scores / ctx / LN stat banks
    e["ps_st"] = e["ps_sc"]
    e["ps_cx"] = e["ps_sc"]

    # ---- constants ----
    identb = e["p_cst"].tile([128, 128], BF16, tag="identb")
    make_identity(nc, identb[:])
    ones128 = e["p_cst"].tile([128, 128], F32R, tag="ones128")
    nc.sync.dma_start(ones128[:], d["ones"].bitcast(F32R))
    eps_t = e["p_cst"].tile([128, 1], F32, tag="eps")
    nc.gpsimd.memset(eps_t[:], EPS)
    e["identb"] = identb
    e["ones128"], e["eps_t"] = ones128, eps_t

    ab_t = e["p_cst"].tile([128, TC], F32, tag="ab")
    nc.sync.dma_start(ab_t[:], d["ab"].rearrange("(c p) -> p c", p=128))
    e["ab_t"] = ab_t

    def colvec(ap_1d, n, tag, pool="p_bias"):
        t = e[pool].tile([128, n], F32, tag="bias")
        nc.sync.dma_start(t[:], ap_1d.rearrange("(c p) -> p c", p=128))
        return t
    e["colvec"] = colvec

    # =============== embedding ===============
    x0 = e["p_big"].tile([128, TC, H], F32, tag="big")
    for c in range(TC):
        idx_t = e["p_sm"].tile([128, 1], I32, tag="sm")
        nc.sync.dma_start(idx_t[:], d["enc"][128 * c:128 * (c + 1), :])
        nc.gpsimd.indirect_dma_start(
            out=x0[:, c, :], out_offset=None, in_=d["word_emb"][:],
            in_offset=bass.IndirectOffsetOnAxis(ap=idx_t[:, :1], axis=0))
    pos_t = e["p_pos"].tile([128, 2, H], BF16, tag="pos")
    nc.gpsimd.dma_start(pos_t[:, 0, :], d["pos_p"][0:128, :])
    nc.gpsimd.dma_start(pos_t[:, 1, :], d["pos_p"][128:256, :])

    xq8 = e["p_x8"].tile([128, HC, T], FP8, tag="x8")
    xh = e["p_xt"].tile([128, HC, T], F32R, tag="xt")
    for c in range(TC):
        xc = x0[:, c, :]
        nc.vector.tensor_tensor(out=xc, in0=xc, in1=pos_t[:, c % 2, :], op=ALU.add)
        # token-major layernorm (per-partition stats)
        su = e["p_sm"].tile([128, 4], F32, tag="sm")
        nc.vector.reduce_sum(out=su[:, 0:1], in_=xc, axis=mybir.AxisListType.X)
        sq = e["p_scr"].tile([128, H], F32, tag="scr")
        nc.scalar.activation(sq[:], xc, AF.Square, accum_out=su[:, 1:2])
        st = e["p_sm"].tile([128, 4], F32, tag="sm")
        nc.vector.tensor_scalar_mul(st[:, 0:1], su[:, 0:1], 1.0 / H)      # mu
        nc.vector.tensor_scalar_mul(st[:, 1:2], su[:, 1:2], 1.0 / H)      # m2
        nc.vector.tensor_tensor(out=st[:, 2:3], in0=st[:, 0:1], in1=st[:, 0:1], op=ALU.mult)
        nc.vector.tensor_tensor(out=st[:, 3:4], in0=st[:, 1:2], in1=st[:, 2:3], op=ALU.subtract)
        sd = e["p_sm"].tile([128, 2], F32, tag="sm")
        nc.scalar.activation(sd[:, 0:1], st[:, 3:4], AF.Ln, bias=eps_t[:, 0:1])
        nc.scalar.activation(sd[:, 1:2], sd[:, 0:1], AF.Exp, scale=-0.5)
        xcr = e["p_scr"].tile([128, H], BF16, tag="scr")
        nc.vector.tensor_scalar(out=xcr[:], in0=xc, scalar1=st[:, 0:1], scalar2=sd[:, 1:2],
                                op0=ALU.subtract, op1=ALU.mult)
        for k in range(HC):
            pt = e["ps_mm"].tile([128, 128], BF16, tag="ps_mm")
            nc.tensor.transpose(pt[:], xcr[:, 128 * k:128 * (k + 1)], identb[:])
            nc.scalar.activation(xh[:, k, 128 * c:128 * (c + 1)], pt[:], AF.Copy)
            nc.vector.tensor_scalar_mul(xq8[:, k, 128 * c:128 * (c + 1)],
                                        xh[:, k, 128 * c:128 * (c + 1)].bitcast(F32),
                                        SX)

    # =============== transformer layers ===============
    for l in range(n_layers):
        xq8, xh = _layer(nc, d, l, xq8, xh, e)

    # =============== head ===============
    _head(nc, d, xh, e)


def _wload(nc, dst_ap, src2d):
    """DMA a [K, F] dram weight into a k-major [128, K/128, F] SBUF AP."""
    nc.gpsimd.dma_start(dst_ap, src2d.rearrange("(k p) f -> p k f", p=128))


def _ln_finish(nc, e, A, mean_ps, sq_ps):
    """Partition-dim LN from broadcast stat psums; returns fp8 xhat*SX tile."""
    musq = e["p_st"].tile([128, T], F32, tag="st")
    nc.scalar.activation(musq[:], mean_ps[:], AF.Square, scale=1.0 / H)
    var = e["p_st"].tile([128, T], F32, tag="st")
    nc.vector.scalar_tensor_tensor(out=var[:], in0=sq_ps[:], scalar=1.0 / H,
                                   in1=musq[:], op0=ALU.mult, op1=ALU.subtract)
    lnv = e["p_st"].tile([128, T], F32, tag="st")
    nc.scalar.activation(lnv[:], var[:], AF.Ln, bias=e["eps_t"][:, 0:1])
    rstd = e["p_st"].tile([128, T], F32, tag="st")
    nc.scalar.activation(rstd[:], lnv[:], AF.Exp, scale=-0.5)
    xq8n = e["p_x8"].tile([128, HC, T], FP8, tag="x8")
    for k in range(HC):
        t2 = e["p_scr"].tile([128, T], F32, tag="scr")
        nc.vector.scalar_tensor_tensor(out=t2[:], in0=mean_ps[:], scalar=-1.0 / H,
                                       in1=A[:, k, :].bitcast(F32),
                                       op0=ALU.mult, op1=ALU.add)
        # xhat overwrites the residual tile in place (read by the y-ops)
        nc.vector.tensor_tensor(out=A[:, k, :], in0=t2[:], in1=rstd[:], op=ALU.mult)
        nc.vector.tensor_scalar_mul(xq8n[:, k, :], A[:, k, :].bitcast(F32), SX)
    return xq8n


def _layer(nc, d, l, xq8, xh_in, e):
    ps_mm, ps_st, ps_sc, ps_cx = e["ps_mm"], e["ps_st"], e["ps_sc"], e["ps_cx"]
    ones128, ab_t = e["ones128"], e["ab_t"]
    colvec = e["colvec"]

    # ---- per-layer weights & biases ----
    wqk = e["p_w4"].tile([128, HC, 2 * H], FP8, tag="w4")
    for i, wn in enumerate(["Wq8", "Wk8"]):
        _wload(nc, wqk[:, :, H * i:H * (i + 1)], d[wn][l])
    wv = e["p_wv"].tile([128, HC, 2 * H], FP8, tag="wv")
    _wload(nc, wv[:, :, 0:H], d["Wv8"][l])
    _wload(nc, wv[:, :, H:2 * H], d["DWv8"][l])
    wo = e["p_wo"].tile([128, HC, H], BF16, tag="wo")
    _wload(nc, wo[:], d["Wob"][l])
    bqk = e["p_bias"].tile([128, 2 * HC], F32, tag="bias")
    nc.sync.dma_start(bqk[:, 0:HC], d["bq_f"][l].rearrange("(c p) -> p c", p=128))
    nc.sync.dma_start(bqk[:, HC:2 * HC], d["bk_f"][l].rearrange("(c p) -> p c", p=128))
    yv = e["p_bias"].tile([128, 4 * HC], F32, tag="bias")
    for i, yn in enumerate(["y2s", "y2b", "y1s", "y1b"]):
        nc.sync.dma_start(yv[:, HC * i:HC * (i + 1)],
                          d[yn][l].rearrange("(c p) -> p c", p=128))
    bi_t = colvec(d["bi_f"][l], FFC, "bi")

    # ---- Q, K projections (dh-major) ----
    QT = e["p_qk"].tile([128, HC, T], BF16, tag="qk")
    KT = e["p_qk"].tile([128, HC, T], BF16, tag="qk")
    for pi, dst in enumerate([QT, KT]):
        for m in range(HC):
            pm_ = ps_mm.tile([128, T], F32, tag="ps_mm")
            for j in range(HC // 2):
                nc.tensor.matmul(
                    pm_[:], wqk[:, 2 * j:2 * j + 2, H * pi + 128 * m:H * pi + 128 * (m + 1)],
                    xq8[:, 2 * j:2 * j + 2, :],
                    start=(j == 0), stop=(j == HC // 2 - 1), perf_mode=PM.DoubleRow)
            nc.scalar.activation(dst[:, m, :], pm_[:], AF.Identity, scale=ISC,
                                 bias=bqk[:, HC * pi + m:HC * pi + m + 1])

    # ---- V projection, token-major, with 1/SX columns for softmax sums ----
    Vt = e["p_v"].tile([128, TC, 65 * NH], FP8, tag="v")
    vones = Vt[:].rearrange("p c (h x) -> p c h x", h=NH)[:, :, :, 64:65]
    nc.gpsimd.memset(vones, VCOL)
    for c in range(TC):
        for hf in range(2):
            pm_ = ps_mm.tile([128, 384], F32, tag="ps_mm")
            for dw in range(2):
                for j in range(HC // 2):
                    nc.tensor.matmul(
                        pm_[:], xq8[:, 2 * j:2 * j + 2, 128 * c:128 * (c + 1)],
                        wv[:, 2 * j:2 * j + 2, H * dw + 384 * hf:H * dw + 384 * (hf + 1)],
                        start=(dw == 0 and j == 0),
                        stop=(dw == 1 and j == HC // 2 - 1), perf_mode=PM.DoubleRow)
            dstv = Vt[:, c, 390 * hf:390 * (hf + 1)].rearrange(
                "p (h x) -> p h x", h=6)[:, :, 0:64]
            nc.scalar.activation(dstv, pm_[:].rearrange("p (h x) -> p h x", h=6),
                                 AF.Copy, scale=ISC)

    # ---- attention ----
    ctok = e["p_c8"].tile([128, TC, H], BF16, tag="c8")
    for s in range(BPC):
        for h in range(NH):
            kc, po = h // 2, 64 * (h % 2)
            exp8 = e["p_exp"].tile([128, 2, 256], FP8, tag="exp")
            for j in range(2):
                psc = ps_sc.tile([128, 256], F32, tag="ps_sc")
                nc.tensor.matmul(
                    psc[:],
                    KT[po:po + 64, kc, 256 * s + 128 * j:256 * s + 128 * (j + 1)],
                    QT[po:po + 64, kc, 256 * s:256 * (s + 1)],
                    start=True, stop=True)
                nc.scalar.activation(exp8[:, j, :], psc[:], AF.Exp, scale=0.125,
                                     bias=ab_t[:, 2 * s + j:2 * s + j + 1])
            for i in range(2):
                pcx = ps_cx.tile([128, 65], F32, tag="ps_sc")
                nc.tensor.matmul(pcx[:], exp8[:, 0:2, 128 * i:128 * (i + 1)],
                                 Vt[:, 2 * s:2 * s + 2, 65 * h:65 * h + 65],
                                 start=True, stop=True, perf_mode=PM.DoubleRow)
                rec = e["p_sm"].tile([128, 1], F32, tag="sm")
                nc.vector.reciprocal(rec[:], pcx[:, 64:65])
                nc.vector.tensor_scalar_mul(
                    ctok[:, 2 * s + i, 64 * h:64 * h + 64], pcx[:, 0:64], rec[:])

    # ---- transpose ctx to [H, T] bf16 ----
    ctxT = e["p_c8"].tile([128, HC, T], BF16, tag="c8")
    for c in range(TC):
        for k in range(HC):
            pt = ps_mm.tile([128, 128], BF16, tag="ps_mm")
            nc.tensor.transpose(pt[:], ctok[:, c, 128 * k:128 * (k + 1)],
                                e["identb"][:])
            nc.scalar.activation(ctxT[:, k, 128 * c:128 * (c + 1)], pt[:], AF.Copy)

    # ---- O projection + residual + LN1 stats ----
    A = e["p_xt"].tile([128, HC, T], F32R, tag="xt")
    mean_ps = ps_st.tile([128, T], F32, tag="ps_sc")
    sq_ps = ps_st.tile([128, T], F32, tag="ps_sc")
    for o in range(HC):
        pm_ = ps_mm.tile([128, T], F32, tag="ps_mm")
        for k in range(HC):
            nc.tensor.matmul(pm_[:], wo[:, k, 128 * o:128 * (o + 1)],
                             ctxT[:, k, :], start=(k == 0), stop=(k == HC - 1))
        y2o = e["p_scr"].tile([128, T], F32, tag="scr")
        nc.scalar.activation(y2o[:], xh_in[:, o, :].bitcast(F32), AF.Identity,
                             scale=yv[:, o:o + 1], bias=yv[:, HC + o:HC + o + 1])
        nc.vector.scalar_tensor_tensor(out=A[:, o, :], in0=pm_[:], scalar=1.0,
                                       in1=y2o[:], op0=ALU.mult, op1=ALU.add)
        nc.tensor.matmul(mean_ps[:], ones128[:], A[:, o, :],
                         start=(o == 0), stop=(o == HC - 1))
        sq = e["p_sq"].tile([128, T], F32R, tag="sq")
        nc.scalar.activation(sq[:], A[:, o, :].bitcast(F32), AF.Square)
        nc.tensor.matmul(sq_ps[:], ones128[:], sq[:],
                         start=(o == 0), stop=(o == HC - 1))
    xq81 = _ln_finish(nc, e, A, mean_ps, sq_ps)


    # ---- FFN: two FF-chunk halves; Wi fp8+delta compensated, Wo2 bf16 ----
    Apre = e["p_xt"].tile([128, HC, T], F32R, tag="xt")
    mean2 = ps_st.tile([128, T], F32, tag="ps_sc")
    sq2 = ps_st.tile([128, T], F32, tag="ps_sc")
    FH = FF // 2                       # 1536 ff cols per half
    KH = FFC // 2                      # 12 ff chunks per half
    for wh in range(2):
        wi = e["p_wi"].tile([128, HC, FF], FP8, tag="wi")
        nc.gpsimd.dma_start(
            wi[:, :, 0:FH], d["Wi8"][l].rearrange("(k p) f -> p k f", p=128)
            [:, :, FH * wh:FH * (wh + 1)])
        nc.gpsimd.dma_start(
            wi[:, :, FH:FF], d["DWi8"][l].rearrange("(k p) f -> p k f", p=128)
            [:, :, FH * wh:FH * (wh + 1)])
        wo2 = e["p_wo2"].tile([128, KH, H], BF16, tag="wo2")
        _wload(nc, wo2[:], d["Wo2b"][l, FH * wh:FH * (wh + 1)])
        gel = e["p_big"].tile([128, KH, T], BF16, tag="big")
        for mi in range(KH):
            m = KH * wh + mi
            pm_ = ps_mm.tile([128, T], F32, tag="ps_mm")
            for dw in range(2):
                for j in range(HC // 2):
                    nc.tensor.matmul(
                        pm_[:], wi[:, 2 * j:2 * j + 2,
                                   FH * dw + 128 * mi:FH * dw + 128 * (mi + 1)],
                        xq81[:, 2 * j:2 * j + 2, :],
                        start=(dw == 0 and j == 0),
                        stop=(dw == 1 and j == HC // 2 - 1), perf_mode=PM.DoubleRow)
            nc.scalar.activation(gel[:, mi, :], pm_[:], AF.Gelu, scale=ISC,
                                 bias=bi_t[:, m:m + 1])
        for o in range(HC):
            pm_ = ps_mm.tile([128, T], F32, tag="ps_mm")
            for k in range(KH):
                nc.tensor.matmul(pm_[:], wo2[:, k, 128 * o:128 * (o + 1)],
                                 gel[:, k, :], start=(k == 0), stop=(k == KH - 1))
            if wh == 0:
                y1o = e["p_scr"].tile([128, T], F32, tag="scr")
                nc.scalar.activation(y1o[:], A[:, o, :].bitcast(F32), AF.Identity,
                                     scale=yv[:, 2 * HC + o:2 * HC + o + 1],
                                     bias=yv[:, 3 * HC + o:3 * HC + o + 1])
                nc.vector.scalar_tensor_tensor(out=Apre[:, o, :], in0=pm_[:],
                                               scalar=1.0, in1=y1o[:],
                                               op0=ALU.mult, op1=ALU.add)
            else:
                nc.vector.scalar_tensor_tensor(out=Apre[:, o, :], in0=pm_[:],
                                               scalar=1.0,
                                               in1=Apre[:, o, :].bitcast(F32),
                                               op0=ALU.mult, op1=ALU.add)
                nc.tensor.matmul(mean2[:], ones128[:], Apre[:, o, :],
                                 start=(o == 0), stop=(o == HC - 1))
                sq = e["p_sq"].tile([128, T], F32R, tag="sq")
                nc.scalar.activation(sq[:], Apre[:, o, :].bitcast(F32), AF.Square)
                nc.tensor.matmul(sq2[:], ones128[:], sq[:],
                                 start=(o == 0), stop=(o == HC - 1))
    return _ln_finish(nc, e, Apre, mean2, sq2), Apre


def _head(nc, d, xh, e):
    ps_mm = e["ps_mm"]
    colvec = e["colvec"]
    hs_t = colvec(d["hs"], HC, "hs")
    hb_t = colvec(d["hb"], HC, "hb")
    # relu(x) with final LN scale/bias fused, bf16
    reluT = e["p_qk"].tile([128, HC, T], BF16, tag="qk")
    for k in range(HC):
        nc.scalar.activation(reluT[:, k, :], xh[:, k, :].bitcast(F32), AF.Relu,
                             scale=hs_t[:, k:k + 1], bias=hb_t[:, k:k + 1])
    # f1 = relu(relu(x) @ w1 + b1), [M1C, T] bf16
    b1_t = colvec(d["b1"], M1C, "b1")
    w1t = e["p_w12"].tile([128, HC, M1], BF16, tag="w12")
    _wload(nc, w1t[:], d["w1b"])
    f1 = e["p_f1"].tile([128, M1C, T], BF16, tag="f1")
    for m in range(M1C):
        pm_ = ps_mm.tile([128, T], F32, tag="ps_mm")
        for k in range(HC):
            nc.tensor.matmul(pm_[:], w1t[:, k, 128 * m:128 * (m + 1)], reluT[:, k, :],
                             start=(k == 0), stop=(k == HC - 1))
        nc.scalar.activation(f1[:, m, :], pm_[:], AF.Relu, bias=b1_t[:, m:m + 1])
    # f2 = f1 @ w2 + b2, token-major [TC, C]
    w2t = e["p_w2h"].tile([128, M1C, C], BF16, tag="w2h")
    _wload(nc, w2t[:], d["w2b"])
    b2bc = e["p_lnbc"].tile([128, C], BF16, tag="lnbc")
    nc.gpsimd.dma_start(b2bc[:], d["b2"][None, :].partition_broadcast(128)[:, 0, :])
    f2 = e["p_f2"].tile([128, TC, CPAD], F32R, tag="f2")
    nc.gpsimd.memset(f2[:].bitcast(F32), 0.0)
    for c in range(TC):
        pm_ = ps_mm.tile([128, C], F32, tag="ps_mm")
        for k in range(M1C):
            nc.tensor.matmul(pm_[:], f1[:, k, 128 * c:128 * (c + 1)], w2t[:, k, :],
                             start=(k == 0), stop=(k == M1C - 1))
        nc.vector.tensor_tensor(out=f2[:, c, 0:C], in0=pm_[:], in1=b2bc[:],
                                op=ALU.add)
    # pooling + final softmax (N padded to 428 for fp32r)
    CP2 = 428
    for s in range(BPC):
        ppool = ps_mm.tile([128, CP2], F32, tag="ps_mm")
        for j in range(2):
            pm_t = e["p_exp"].tile([128, 128], F32R, tag="exp")
            nc.sync.dma_start(pm_t[:], d["pmat"][256 * s + 128 * j:256 * s + 128 * (j + 1), :].bitcast(F32R))
            nc.tensor.matmul(ppool[:], pm_t[:], f2[:, 2 * s + j, 0:CP2],
                             start=(j == 0), stop=(j == 1))
        for half, src in ((0, ppool[:, 0:C]), (1, f2[:, 2 * s + 1, 0:C].bitcast(F32))):
            ex = e["p_scr"].tile([128, CPAD], F32, tag="scr")
            se = e["p_sm"].tile([128, 2], F32, tag="sm")
            nc.scalar.activation(ex[:, 0:C], src, AF.Exp, accum_out=se[:, 0:1])
            nc.vector.reciprocal(se[:, 1:2], se[:, 0:1])
            nc.vector.tensor_scalar_mul(ex[:, 0:C], ex[:, 0:C], se[:, 1:2])
            row0 = 256 * s + 128 * half
            nc.sync.dma_start(d["out_d"][row0:row0 + 128, :], ex[:, 0:C])


# ======================= host side =======================

_PROG_CACHE = {}


def _get_program(n_layers=L):
    if n_layers not in _PROG_CACHE:
        _PROG_CACHE[n_layers] = build_program(n_layers)
    return _PROG_CACHE[n_layers]


def make_in_maps(inputs, n_layers=L):
    """Build per-core input maps; fold LN scale/bias into weights, pre-cast
    weights to fp8/bf16."""
    f32 = lambda x: np.ascontiguousarray(np.asarray(x), dtype=np.float32)
    fp8 = lambda x: np.ascontiguousarray(
        np.asarray(x, dtype=np.float32).astype(ml_dtypes.float8_e4m3fn))
    bf16 = lambda x: np.ascontiguousarray(
        np.asarray(x, dtype=np.float32).astype(ml_dtypes.bfloat16))
    enc = np.asarray(inputs["encoded_batch"], dtype=np.int32)
    mask = np.asarray(inputs["mask"], dtype=np.int32)
    wpt = np.asarray(inputs["word_piece_tracked"], dtype=np.int32)

    Wq, Wk, Wv, Wo = (f32(inputs[k]) for k in ["Wq", "Wk", "Wv", "Wo"])
    Wi, Wo2 = f32(inputs["Wi"]), f32(inputs["Wo2"])
    bq, bk, bv, bo = (f32(inputs[k]) for k in ["bq", "bk", "bv", "bo"])
    bi, bo2 = f32(inputs["bi"]), f32(inputs["bo2"])
    ln1_s, ln1_b = f32(inputs["ln1_s"]), f32(inputs["ln1_b"])
    ln2_s, ln2_b = f32(inputs["ln2_s"]), f32(inputs["ln2_b"])

    s_in = np.empty((L, H), np.float32)
    b_in = np.empty((L, H), np.float32)
    s_in[0], b_in[0] = f32(inputs["emb_ln_s"]), f32(inputs["emb_ln_b"])
    s_in[1:], b_in[1:] = ln2_s[:L - 1], ln2_b[:L - 1]

    Wq8 = np.empty((L, H, H), ml_dtypes.float8_e4m3fn)
    Wk8 = np.empty_like(Wq8)
    Wv8 = np.empty_like(Wq8)
    Wi8 = np.empty((L, H, FF), ml_dtypes.float8_e4m3fn)
    bq_f = np.empty((L, H), np.float32)
    bk_f = np.empty_like(bq_f)
    y2b = np.empty_like(bq_f)
    bi_f = np.empty((L, FF), np.float32)
    DWv8 = np.empty_like(Wq8)
    DWi8 = np.empty_like(Wi8)
    for l in range(L):
        Wq8[l] = fp8(s_in[l][:, None] * Wq[l] * SW)
        Wk8[l] = fp8(s_in[l][:, None] * Wk[l] * SW)
        wv_t = s_in[l][:, None] * Wv[l] * SW
        Wv8[l] = fp8(wv_t)
        DWv8[l] = fp8(wv_t - Wv8[l].astype(np.float32))
        wi_t = ln1_s[l][:, None] * Wi[l] * SW
        Wi8[l] = fp8(wi_t)
        DWi8[l] = fp8(wi_t - Wi8[l].astype(np.float32))
        bq_f[l] = b_in[l] @ Wq[l] + bq[l]
        bk_f[l] = b_in[l] @ Wk[l] + bk[l]
        bv_full = b_in[l] @ Wv[l] + bv[l]
        bo_full = bv_full @ Wo[l] + bo[l]
        y2b[l] = b_in[l] + bo_full
        bi_f[l] = b_in_ff = ln1_b[l] @ Wi[l] + bi[l]

    # pooling matrix P[b, s, w] = 1/cnt[b,w] if seg[b,s]==w else 0
    cum = np.cumsum(wpt, axis=1)
    P = np.zeros((B, S, W), dtype=np.float32)
    for b in range(B):
        seg = np.searchsorted(cum[b], np.arange(S), side="right")
        valid = seg < W
        P[b, np.arange(S)[valid], seg[valid]] = 1.0 / wpt[b, seg[valid]]

    ab = (1.0 - mask.astype(np.float32)) * -10000.0

    rep = dict(
        word_emb=f32(inputs["word_emb"]),
        pos_p=f32(inputs["pos_emb"]) + f32(inputs["type_emb"])[0][None, :],
        Wq8=Wq8, Wk8=Wk8, Wv8=Wv8, DWv8=DWv8, Wob=bf16(Wo),
        Wi8=Wi8, DWi8=DWi8, Wo2b=bf16(Wo2),
        bq_f=bq_f, bk_f=bk_f, bi_f=bi_f,
        y2s=s_in, y2b=y2b,
        y1s=ln1_s, y1b=ln1_b + bo2,
        hs=ln2_s[n_layers - 1], hb=ln2_b[n_layers - 1],
        w1b=bf16(inputs["w1"]), b1=f32(inputs["b1"]),
        w2b=bf16(inputs["w2"]), b2=f32(inputs["b2"]),
        ones=np.ones((128, 128), np.float32),
    )

    in_maps = []
    for core in range(N_CORES):
        b0 = core * BPC
        m = dict(rep)
        m["enc"] = enc[b0:b0 + BPC].reshape(T, 1)
        m["ab"] = ab[b0:b0 + BPC].reshape(T)
        m["pmat"] = P[b0:b0 + BPC].reshape(T, W)
        in_maps.append(m)
    return in_maps


def kernel(**inputs):
    nc = _get_program(L)
    in_maps = make_in_maps(inputs, L)
    res = run_bass_kernel_spmd(nc, in_maps, core_ids=list(range(N_CORES)))
    out = np.concatenate([res.results[i]["out"].reshape(BPC, S, C)
                          for i in range(N_CORES)], axis=0)
    return out.astype(np.float32)


# revision 34
# speedup vs baseline: 1.2940x; 1.0586x over previous
"""Trainium2 Bass kernel for the CCG supertagger BERT model.

Data-parallel over batch: 16 samples -> 8 cores x 2 samples.
Key optimizations over the v1 kernel:
  - fp8e4m3 DoubleRow matmuls (0.5 cyc/row) for QKV, V, Wo, Wi, Wo2.
  - LayerNorm scale/bias folded into the next matmul's weights/biases on
    the host; residual re-applications via per-partition scalar ops.
  - LN stats broadcast across partitions by using a [128,128] ones
    stationary (cost only depends on moving rows), rstd computed as
    exp(-0.5*ln(var+eps)) on the scalar engine (stays in the exp table).
  - V projected directly token-major (no PE transposes); softmax sums
    folded into the ctx matmul via a 1/16-column appended to V.
  - All weights pre-cast to fp8/bf16 on the host (4x less HBM traffic).
"""
import numpy as np
import ml_dtypes

import concourse.bass as bass
import concourse.tile as tile
from concourse import bacc, mybir
from concourse.bass_utils import run_bass_kernel_spmd
from concourse.masks import make_identity

F32 = mybir.dt.float32
F32R = mybir.dt.float32r
BF16 = mybir.dt.bfloat16
FP8 = mybir.dt.float8e4
I32 = mybir.dt.int32
AF = mybir.ActivationFunctionType
ALU = mybir.AluOpType
PM = mybir.MatmulPerfMode

B, S, W = 16, 256, 128
V, H, L, NH, DH, FF, C = 30522, 768, 12, 12, 64, 3072, 425
EPS = 1e-12
N_CORES = 8
BPC = B // N_CORES          # samples per core
T = BPC * S                 # tokens per core (512)
HC = H // 128               # 6
FFC = FF // 128             # 24
TC = T // 128               # 4 token chunks
M1 = 1024
M1C = M1 // 128             # 8
CPAD = 448                  # padded class dim for sbuf tiles
SX = 16.0                   # fp8 activation scale
SW = 64.0                   # fp8 weight scale
ISC = 1.0 / (SX * SW)       # psum descale for fp8 x fp8 matmuls
VCOL = 1.0                  # value of the ones-column appended to V


def build_program(n_layers=L):
    nc = bacc.Bacc("TRN2", target_bir_lowering=False, debug=False,
                   num_devices=N_CORES)

    dt_ = lambda name, shape, dt, kind: nc.dram_tensor(name, shape, dt, kind=kind).ap()
    d = {}
    # per-core sharded inputs
    d["enc"] = dt_("enc", [T, 1], I32, "ExternalInput")
    d["ab"] = dt_("ab", [T], F32, "ExternalInput")          # attn bias per key pos
    d["pmat"] = dt_("pmat", [T, 128], F32, "ExternalInput")  # pooling matrices
    # replicated model inputs (host pre-processed)
    d["word_emb"] = dt_("word_emb", [V, H], F32, "ExternalInput")
    d["pos_p"] = dt_("pos_p", [S, H], F32, "ExternalInput")  # pos + type emb
    d["Wq8"] = dt_("Wq8", [L, H, H], FP8, "ExternalInput")
    d["Wk8"] = dt_("Wk8", [L, H, H], FP8, "ExternalInput")
    d["Wv8"] = dt_("Wv8", [L, H, H], FP8, "ExternalInput")
    d["DWv8"] = dt_("DWv8", [L, H, H], FP8, "ExternalInput")
    d["DWi8"] = dt_("DWi8", [L, H, FF], FP8, "ExternalInput")
    d["Wob"] = dt_("Wob", [L, H, H], BF16, "ExternalInput")
    d["Wi8"] = dt_("Wi8", [L, H, FF], FP8, "ExternalInput")
    d["Wo2b"] = dt_("Wo2b", [L, FF, H], BF16, "ExternalInput")
    d["bq_f"] = dt_("bq_f", [L, H], F32, "ExternalInput")
    d["bk_f"] = dt_("bk_f", [L, H], F32, "ExternalInput")
    d["bi_f"] = dt_("bi_f", [L, FF], F32, "ExternalInput")
    d["y2s"] = dt_("y2s", [L, H], F32, "ExternalInput")   # s_in/SX
    d["y2b"] = dt_("y2b", [L, H], F32, "ExternalInput")   # b_in + bo_full
    d["y1s"] = dt_("y1s", [L, H], F32, "ExternalInput")   # ln1_s/SX
    d["y1b"] = dt_("y1b", [L, H], F32, "ExternalInput")   # ln1_b + bo2
    d["hs"] = dt_("hs", [H], F32, "ExternalInput")        # ln2_s[-1]/SX
    d["hb"] = dt_("hb", [H], F32, "ExternalInput")        # ln2_b[-1]
    d["w1b"] = dt_("w1b", [H, M1], BF16, "ExternalInput")
    d["b1"] = dt_("b1", [M1], F32, "ExternalInput")
    d["w2b"] = dt_("w2b", [M1, C], BF16, "ExternalInput")
    d["b2"] = dt_("b2", [C], F32, "ExternalInput")
    d["ones"] = dt_("ones", [128, 128], F32, "ExternalInput")
    d["out_d"] = dt_("out", [T, C], F32, "ExternalOutput")

    with tile.TileContext(nc) as tc:
        from contextlib import ExitStack
        with ExitStack() as ctx:
            _emit(nc, tc, n_layers, d, ctx)
    nc.compile()
    return nc


def _emit(nc, tc, n_layers, d, ctx):
    pool = lambda name, bufs, space="SBUF": ctx.enter_context(
        tc.tile_pool(name=name, bufs=bufs, space=space))

    e = {}
    e["p_x8"] = pool("x8", 3)       # [128, HC, T] fp8 normalized acts (xhat*SX)
    e["p_xt"] = pool("xt", 2)       # [128, HC, T] f32r residual stream
    e["p_qk"] = pool("qk", 2)       # [128, HC, T] bf16 (QT, KT)
    e["p_v"] = pool("v", 1)         # [128, TC, 780] fp8 token-major V + ones cols
    e["p_c8"] = pool("c8", 2)       # [128, TC, H] bf16 ctok / [128, HC, T] fp8 ctxT
    e["p_exp"] = pool("exp", 3)     # [128, 256] bf16 exp tiles
    e["p_big"] = pool("big", 1)     # 12KB arena: emb x0 f32 / gel8 fp8
    e["p_scr"] = pool("scr", 3)     # [128, T] f32 scratch (t2, y2', y1')
    e["p_st"] = pool("st", 3)       # [128, T] f32 LN stats (musq/var/lnv/rstd)
    e["p_sq"] = pool("sq", 2)       # [128, T] f32r square scratch
    e["p_w4"] = pool("w4", 2)       # [128, HC, 2H] fp8 Wq|Wk
    e["p_wv"] = pool("wv", 1)       # [128, HC, 2H] fp8 Wv|DWv
    e["p_wo"] = pool("wo", 1)       # [128, HC, H] bf16 Wo
    e["p_wi"] = pool("wi", 1)       # [128, HC, FF] fp8 (Wi|DWi half)
    e["p_wo2"] = pool("wo2", 1)     # [128, FFC/2, H] bf16 Wo2 half
    e["p_w12"] = pool("w12", 1)     # [128, HC, M1] fp8 w1
    e["p_w2h"] = pool("w2h", 1)     # [128, M1C, C] fp8 w2
    e["p_f1"] = pool("f1", 1)       # [128, M1C, T] fp8
    e["p_f2"] = pool("f2", 1)       # [128, TC, CPAD] f32r
    e["p_pos"] = pool("pos", 1)     # [128, 2, H] bf16 pos embedding
    e["p_bias"] = pool("bias", 6)   # small per-layer bias tiles
    e["p_sm"] = pool("sm", 4)       # small scalars
    e["p_cst"] = pool("cst", 1)     # constants
    e["p_lnbc"] = pool("lnbc", 1)   # [128, C] bf16 b2 broadcast

    e["ps_mm"] = pool("ps_mm", 4, "PSUM")   # [128, 512] f32 main matmul banks
    e["ps_sc"] = pool("ps_sc", 4, "PSUM")   # scores / ctx / LN stat banks
    e["ps_st"] = e["ps_sc"]
    e["ps_cx"] = e["ps_sc"]

    # ---- constants ----
    identb = e["p_cst"].tile([128, 128], BF16, tag="identb")
    make_identity(nc, identb[:])
    ones128 = e["p_cst"].tile([128, 128], F32R, tag="ones128")
    nc.sync.dma_start(ones128[:], d["ones"].bitcast(F32R))
    eps_t = e["p_cst"].tile([128, 1], F32, tag="eps")
    nc.gpsimd.memset(eps_t[:], EPS)
    e["identb"] = identb
    e["ones128"], e["eps_t"] = ones128, eps_t

    ab_t = e["p_cst"].tile([128, TC], F32, tag="ab")
    nc.sync.dma_start(ab_t[:], d["ab"].rearrange("(c p) -> p c", p=128))
    e["ab_t"] = ab_t

    def colvec(ap_1d, n, tag, pool="p_bias"):
        t = e[pool].tile([128, n], F32, tag="bias")
        nc.sync.dma_start(t[:], ap_1d.rearrange("(c p) -> p c", p=128))
        return t
    e["colvec"] = colvec

    # =============== embedding ===============
    x0 = e["p_big"].tile([128, TC, H], F32, tag="big")
    for c in range(TC):
        idx_t = e["p_sm"].tile([128, 1], I32, tag="sm")
        nc.sync.dma_start(idx_t[:], d["enc"][128 * c:128 * (c + 1), :])
        nc.gpsimd.indirect_dma_start(
            out=x0[:, c, :], out_offset=None, in_=d["word_emb"][:],
            in_offset=bass.IndirectOffsetOnAxis(ap=idx_t[:, :1], axis=0))
    pos_t = e["p_pos"].tile([128, 2, H], BF16, tag="pos")
    nc.gpsimd.dma_start(pos_t[:, 0, :], d["pos_p"][0:128, :])
    nc.gpsimd.dma_start(pos_t[:, 1, :], d["pos_p"][128:256, :])

    xq8 = e["p_x8"].tile([128, HC, T], FP8, tag="x8")
    xh = e["p_xt"].tile([128, HC, T], F32R, tag="xt")
    for c in range(TC):
        xc = x0[:, c, :]
        nc.vector.tensor_tensor(out=xc, in0=xc, in1=pos_t[:, c % 2, :], op=ALU.add)
        # token-major layernorm (per-partition stats)
        su = e["p_sm"].tile([128, 4], F32, tag="sm")
        nc.vector.reduce_sum(out=su[:, 0:1], in_=xc, axis=mybir.AxisListType.X)
        sq = e["p_scr"].tile([128, H], F32, tag="scr")
        nc.scalar.activation(sq[:], xc, AF.Square, accum_out=su[:, 1:2])
        st = e["p_sm"].tile([128, 4], F32, tag="sm")
        nc.vector.tensor_scalar_mul(st[:, 0:1], su[:, 0:1], 1.0 / H)      # mu
        nc.vector.tensor_scalar_mul(st[:, 1:2], su[:, 1:2], 1.0 / H)      # m2
        nc.vector.tensor_tensor(out=st[:, 2:3], in0=st[:, 0:1], in1=st[:, 0:1], op=ALU.mult)
        nc.vector.tensor_tensor(out=st[:, 3:4], in0=st[:, 1:2], in1=st[:, 2:3], op=ALU.subtract)
        sd = e["p_sm"].tile([128, 2], F32, tag="sm")
        nc.scalar.activation(sd[:, 0:1], st[:, 3:4], AF.Ln, bias=eps_t[:, 0:1])
        nc.scalar.activation(sd[:, 1:2], sd[:, 0:1], AF.Exp, scale=-0.5)
        xcr = e["p_scr"].tile([128, H], BF16, tag="scr")
        nc.vector.tensor_scalar(out=xcr[:], in0=xc, scalar1=st[:, 0:1], scalar2=sd[:, 1:2],
                                op0=ALU.subtract, op1=ALU.mult)
        for k in range(HC):
            pt = e["ps_mm"].tile([128, 128], BF16, tag="ps_mm")
            nc.tensor.transpose(pt[:], xcr[:, 128 * k:128 * (k + 1)], identb[:])
            nc.scalar.activation(xh[:, k, 128 * c:128 * (c + 1)], pt[:], AF.Copy)
            nc.vector.tensor_scalar_mul(xq8[:, k, 128 * c:128 * (c + 1)],
                                        xh[:, k, 128 * c:128 * (c + 1)].bitcast(F32),
                                        SX)

    # =============== transformer layers ===============
    for l in range(n_layers):
        xq8, xh = _layer(nc, d, l, xq8, xh, e)

    # =============== head ===============
    _head(nc, d, xh, e)


def _wload(nc, dst_ap, src2d):
    """DMA a [K, F] dram weight into a k-major [128, K/128, F] SBUF AP."""
    nc.gpsimd.dma_start(dst_ap, src2d.rearrange("(k p) f -> p k f", p=128))


def _ln_finish(nc, e, A, mean_ps, sq_ps):
    """Partition-dim LN from broadcast stat psums; returns fp8 xhat*SX tile."""
    musq = e["p_st"].tile([128, T], F32, tag="st")
    nc.scalar.activation(musq[:], mean_ps[:], AF.Square, scale=1.0 / H)
    var = e["p_st"].tile([128, T], F32, tag="st")
    nc.vector.scalar_tensor_tensor(out=var[:], in0=sq_ps[:], scalar=1.0 / H,
                                   in1=musq[:], op0=ALU.mult, op1=ALU.subtract)
    lnv = e["p_st"].tile([128, T], F32, tag="st")
    nc.scalar.activation(lnv[:], var[:], AF.Ln, bias=e["eps_t"][:, 0:1])
    rstd = e["p_st"].tile([128, T], F32, tag="st")
    nc.scalar.activation(rstd[:], lnv[:], AF.Exp, scale=-0.5)
    xq8n = e["p_x8"].tile([128, HC, T], FP8, tag="x8")
    for k in range(HC):
        t2 = e["p_scr"].tile([128, T], F32, tag="scr")
        nc.vector.scalar_tensor_tensor(out=t2[:], in0=mean_ps[:], scalar=-1.0 / H,
                                       in1=A[:, k, :].bitcast(F32),
                                       op0=ALU.mult, op1=ALU.add)
        # xhat overwrites the residual tile in place (read by the y-ops)
        nc.vector.tensor_tensor(out=A[:, k, :], in0=t2[:], in1=rstd[:], op=ALU.mult)
        nc.vector.tensor_scalar_mul(xq8n[:, k, :], A[:, k, :].bitcast(F32), SX)
    return xq8n


def _layer(nc, d, l, xq8, xh_in, e):
    ps_mm, ps_st, ps_sc, ps_cx = e["ps_mm"], e["ps_st"], e["ps_sc"], e["ps_cx"]
    ones128, ab_t = e["ones128"], e["ab_t"]
    colvec = e["colvec"]

    # ---- per-layer weights & biases ----
    wqk = e["p_w4"].tile([128, HC, 2 * H], FP8, tag="w4")
    for i, wn in enumerate(["Wq8", "Wk8"]):
        _wload(nc, wqk[:, :, H * i:H * (i + 1)], d[wn][l])
    wv = e["p_wv"].tile([128, HC, 2 * H], FP8, tag="wv")
    _wload(nc, wv[:, :, 0:H], d["Wv8"][l])
    _wload(nc, wv[:, :, H:2 * H], d["DWv8"][l])
    wo = e["p_wo"].tile([128, HC, H], BF16, tag="wo")
    _wload(nc, wo[:], d["Wob"][l])
    bqk = e["p_bias"].tile([128, 2 * HC], F32, tag="bias")
    nc.sync.dma_start(bqk[:, 0:HC], d["bq_f"][l].rearrange("(c p) -> p c", p=128))
    nc.sync.dma_start(bqk[:, HC:2 * HC], d["bk_f"][l].rearrange("(c p) -> p c", p=128))
    yv = e["p_bias"].tile([128, 4 * HC], F32, tag="bias")
    for i, yn in enumerate(["y2s", "y2b", "y1s", "y1b"]):
        nc.sync.dma_start(yv[:, HC * i:HC * (i + 1)],
                          d[yn][l].rearrange("(c p) -> p c", p=128))
    bi_t = colvec(d["bi_f"][l], FFC, "bi")

    # ---- Q, K projections (dh-major) ----
    QT = e["p_qk"].tile([128, HC, T], BF16, tag="qk")
    KT = e["p_qk"].tile([128, HC, T], BF16, tag="qk")
    for pi, dst in enumerate([QT, KT]):
        for m in range(HC):
            pm_ = ps_mm.tile([128, T], F32, tag="ps_mm")
            for j in range(HC // 2):
                nc.tensor.matmul(
                    pm_[:], wqk[:, 2 * j:2 * j + 2, H * pi + 128 * m:H * pi + 128 * (m + 1)],
                    xq8[:, 2 * j:2 * j + 2, :],
                    start=(j == 0), stop=(j == HC // 2 - 1), perf_mode=PM.DoubleRow)
            nc.scalar.activation(dst[:, m, :], pm_[:], AF.Identity, scale=ISC,
                                 bias=bqk[:, HC * pi + m:HC * pi + m + 1])

    # ---- V projection, token-major, with 1/SX columns for softmax sums ----
    Vt = e["p_v"].tile([128, TC, 65 * NH], FP8, tag="v")
    vones = Vt[:].rearrange("p c (h x) -> p c h x", h=NH)[:, :, :, 64:65]
    nc.gpsimd.memset(vones, VCOL)
    for c in range(TC):
        for hf in range(2):
            pm_ = ps_mm.tile([128, 384], F32, tag="ps_mm")
            for dw in range(2):
                for j in range(HC // 2):
                    nc.tensor.matmul(
                        pm_[:], xq8[:, 2 * j:2 * j + 2, 128 * c:128 * (c + 1)],
                        wv[:, 2 * j:2 * j + 2, H * dw + 384 * hf:H * dw + 384 * (hf + 1)],
                        start=(dw == 0 and j == 0),
                        stop=(dw == 1 and j == HC // 2 - 1), perf_mode=PM.DoubleRow)
            dstv = Vt[:, c, 390 * hf:390 * (hf + 1)].rearrange(
                "p (h x) -> p h x", h=6)[:, :, 0:64]
            nc.scalar.activation(dstv, pm_[:].rearrange("p (h x) -> p h x", h=6),
                                 AF.Copy, scale=ISC)

    # ---- attention ----
    ctok = e["p_c8"].tile([128, TC, H], BF16, tag="c8")
    pend = []

    def emit_ctx(s, h, exp8):
        for i in range(2):
            pcx = ps_cx.tile([128, 65], F32, tag="ps_sc")
            nc.tensor.matmul(pcx[:], exp8[:, 0:2, 128 * i:128 * (i + 1)],
                             Vt[:, 2 * s:2 * s + 2, 65 * h:65 * h + 65],
                             start=True, stop=True, perf_mode=PM.DoubleRow)
            rec = e["p_sm"].tile([128, 1], F32, tag="sm")
            nc.vector.reciprocal(rec[:], pcx[:, 64:65])
            nc.vector.tensor_scalar_mul(
                ctok[:, 2 * s + i, 64 * h:64 * h + 64], pcx[:, 0:64], rec[:])

    for s in range(BPC):
        for h in range(NH):
            kc, po = h // 2, 64 * (h % 2)
            exp8 = e["p_exp"].tile([128, 2, 256], FP8, tag="exp")
            for j in range(2):
                psc = ps_sc.tile([128, 256], F32, tag="ps_sc")
                nc.tensor.matmul(
                    psc[:],
                    KT[po:po + 64, kc, 256 * s + 128 * j:256 * s + 128 * (j + 1)],
                    QT[po:po + 64, kc, 256 * s:256 * (s + 1)],
                    start=True, stop=True)
                nc.scalar.activation(exp8[:, j, :], psc[:], AF.Exp, scale=0.125,
                                     bias=ab_t[:, 2 * s + j:2 * s + j + 1])
            pend.append((s, h, exp8))
            if len(pend) > 1:
                emit_ctx(*pend.pop(0))
    while pend:
        emit_ctx(*pend.pop(0))

    # ---- transpose ctx to [H, T] bf16 ----
    ctxT = e["p_c8"].tile([128, HC, T], BF16, tag="c8")
    for c in range(TC):
        for k in range(HC):
            pt = ps_mm.tile([128, 128], BF16, tag="ps_mm")
            nc.tensor.transpose(pt[:], ctok[:, c, 128 * k:128 * (k + 1)],
                                e["identb"][:])
            nc.scalar.activation(ctxT[:, k, 128 * c:128 * (c + 1)], pt[:], AF.Copy)

    # ---- O projection + residual + LN1 stats ----
    A = e["p_xt"].tile([128, HC, T], F32R, tag="xt")
    mean_ps = ps_st.tile([128, T], F32, tag="ps_sc")
    sq_ps = ps_st.tile([128, T], F32, tag="ps_sc")
    sq_q = []
    for o in range(HC):
        pm_ = ps_mm.tile([128, T], F32, tag="ps_mm")
        for k in range(HC):
            nc.tensor.matmul(pm_[:], wo[:, k, 128 * o:128 * (o + 1)],
                             ctxT[:, k, :], start=(k == 0), stop=(k == HC - 1))
        y2o = e["p_scr"].tile([128, T], F32, tag="scr")
        nc.scalar.activation(y2o[:], xh_in[:, o, :].bitcast(F32), AF.Identity,
                             scale=yv[:, o:o + 1], bias=yv[:, HC + o:HC + o + 1])
        nc.vector.scalar_tensor_tensor(out=A[:, o, :], in0=pm_[:], scalar=1.0,
                                       in1=y2o[:], op0=ALU.mult, op1=ALU.add)
        sq = e["p_sq"].tile([128, T], F32R, tag="sq")
        nc.scalar.activation(sq[:], A[:, o, :].bitcast(F32), AF.Square)
        sq_q.append((o, sq))
        if o > 0:
            nc.tensor.matmul(mean_ps[:], ones128[:], A[:, o - 1, :],
                             start=(o - 1 == 0), stop=False)
            po, psq = sq_q.pop(0)
            nc.tensor.matmul(sq_ps[:], ones128[:], psq[:],
                             start=(po == 0), stop=False)
    nc.tensor.matmul(mean_ps[:], ones128[:], A[:, HC - 1, :],
                     start=False, stop=True)
    po, psq = sq_q.pop(0)
    nc.tensor.matmul(sq_ps[:], ones128[:], psq[:], start=False, stop=True)
    xq81 = _ln_finish(nc, e, A, mean_ps, sq_ps)


    # ---- FFN: two FF-chunk halves; Wi fp8+delta compensated, Wo2 bf16 ----
    Apre = e["p_xt"].tile([128, HC, T], F32R, tag="xt")
    mean2 = ps_st.tile([128, T], F32, tag="ps_sc")
    sq2 = ps_st.tile([128, T], F32, tag="ps_sc")
    FH = FF // 2                       # 1536 ff cols per half
    KH = FFC // 2                      # 12 ff chunks per half
    sq_q = []
    for wh in range(2):
        wi = e["p_wi"].tile([128, HC, FF], FP8, tag="wi")
        nc.gpsimd.dma_start(
            wi[:, :, 0:FH], d["Wi8"][l].rearrange("(k p) f -> p k f", p=128)
            [:, :, FH * wh:FH * (wh + 1)])
        nc.gpsimd.dma_start(
            wi[:, :, FH:FF], d["DWi8"][l].rearrange("(k p) f -> p k f", p=128)
            [:, :, FH * wh:FH * (wh + 1)])
        wo2 = e["p_wo2"].tile([128, KH, H], BF16, tag="wo2")
        _wload(nc, wo2[:], d["Wo2b"][l, FH * wh:FH * (wh + 1)])
        gel = e["p_big"].tile([128, KH, T], BF16, tag="big")
        for mi in range(KH):
            m = KH * wh + mi
            pm_ = ps_mm.tile([128, T], F32, tag="ps_mm")
            for dw in range(2):
                for j in range(HC // 2):
                    nc.tensor.matmul(
                        pm_[:], wi[:, 2 * j:2 * j + 2,
                                   FH * dw + 128 * mi:FH * dw + 128 * (mi + 1)],
                        xq81[:, 2 * j:2 * j + 2, :],
                        start=(dw == 0 and j == 0),
                        stop=(dw == 1 and j == HC // 2 - 1), perf_mode=PM.DoubleRow)
            nc.scalar.activation(gel[:, mi, :], pm_[:], AF.Gelu, scale=ISC,
                                 bias=bi_t[:, m:m + 1])
        for o in range(HC):
            pm_ = ps_mm.tile([128, T], F32, tag="ps_mm")
            for k in range(KH):
                nc.tensor.matmul(pm_[:], wo2[:, k, 128 * o:128 * (o + 1)],
                                 gel[:, k, :], start=(k == 0), stop=(k == KH - 1))
            if wh == 0:
                y1o = e["p_scr"].tile([128, T], F32, tag="scr")
                nc.scalar.activation(y1o[:], A[:, o, :].bitcast(F32), AF.Identity,
                                     scale=yv[:, 2 * HC + o:2 * HC + o + 1],
                                     bias=yv[:, 3 * HC + o:3 * HC + o + 1])
                nc.vector.scalar_tensor_tensor(out=Apre[:, o, :], in0=pm_[:],
                                               scalar=1.0, in1=y1o[:],
                                               op0=ALU.mult, op1=ALU.add)
            else:
                nc.vector.scalar_tensor_tensor(out=Apre[:, o, :], in0=pm_[:],
                                               scalar=1.0,
                                               in1=Apre[:, o, :].bitcast(F32),
                                               op0=ALU.mult, op1=ALU.add)
                sq = e["p_sq"].tile([128, T], F32R, tag="sq")
                nc.scalar.activation(sq[:], Apre[:, o, :].bitcast(F32), AF.Square)
                sq_q.append((o, sq))
                if o > 0:
                    nc.tensor.matmul(mean2[:], ones128[:], Apre[:, o - 1, :],
                                     start=(o - 1 == 0), stop=False)
                    po, psq = sq_q.pop(0)
                    nc.tensor.matmul(sq2[:], ones128[:], psq[:],
                                     start=(po == 0), stop=False)
    nc.tensor.matmul(mean2[:], ones128[:], Apre[:, HC - 1, :],
                     start=False, stop=True)
    po, psq = sq_q.pop(0)
    nc.tensor.matmul(sq2[:], ones128[:], psq[:], start=False, stop=True)
    return _ln_finish(nc, e, Apre, mean2, sq2), Apre


def _head(nc, d, xh, e):
    ps_mm = e["ps_mm"]
    colvec = e["colvec"]
    hs_t = colvec(d["hs"], HC, "hs")
    hb_t = colvec(d["hb"], HC, "hb")
    # relu(x) with final LN scale/bias fused, bf16
    reluT = e["p_qk"].tile([128, HC, T], BF16, tag="qk")
    for k in range(HC):
        nc.scalar.activation(reluT[:, k, :], xh[:, k, :].bitcast(F32), AF.Relu,
                             scale=hs_t[:, k:k + 1], bias=hb_t[:, k:k + 1])
    # f1 = relu(relu(x) @ w1 + b1), [M1C, T] bf16
    b1_t = colvec(d["b1"], M1C, "b1")
    w1t = e["p_w12"].tile([128, HC, M1], BF16, tag="w12")
    _wload(nc, w1t[:], d["w1b"])
    f1 = e["p_f1"].tile([128, M1C, T], BF16, tag="f1")
    for m in range(M1C):
        pm_ = ps_mm.tile([128, T], F32, tag="ps_mm")
        for k in range(HC):
            nc.tensor.matmul(pm_[:], w1t[:, k, 128 * m:128 * (m + 1)], reluT[:, k, :],
                             start=(k == 0), stop=(k == HC - 1))
        nc.scalar.activation(f1[:, m, :], pm_[:], AF.Relu, bias=b1_t[:, m:m + 1])
    # f2 = f1 @ w2 + b2, token-major [TC, C]
    w2t = e["p_w2h"].tile([128, M1C, C], BF16, tag="w2h")
    _wload(nc, w2t[:], d["w2b"])
    b2bc = e["p_lnbc"].tile([128, C], BF16, tag="lnbc")
    nc.gpsimd.dma_start(b2bc[:], d["b2"][None, :].partition_broadcast(128)[:, 0, :])
    f2 = e["p_f2"].tile([128, TC, CPAD], F32R, tag="f2")
    nc.gpsimd.memset(f2[:].bitcast(F32), 0.0)
    for c in range(TC):
        pm_ = ps_mm.tile([128, C], F32, tag="ps_mm")
        for k in range(M1C):
            nc.tensor.matmul(pm_[:], f1[:, k, 128 * c:128 * (c + 1)], w2t[:, k, :],
                             start=(k == 0), stop=(k == M1C - 1))
        nc.vector.tensor_tensor(out=f2[:, c, 0:C], in0=pm_[:], in1=b2bc[:],
                                op=ALU.add)
    # pooling + final softmax (N padded to 428 for fp32r)
    CP2 = 428
    for s in range(BPC):
        ppool = ps_mm.tile([128, CP2], F32, tag="ps_mm")
        for j in range(2):
            pm_t = e["p_exp"].tile([128, 128], F32R, tag="exp")
            nc.sync.dma_start(pm_t[:], d["pmat"][256 * s + 128 * j:256 * s + 128 * (j + 1), :].bitcast(F32R))
            nc.tensor.matmul(ppool[:], pm_t[:], f2[:, 2 * s + j, 0:CP2],
                             start=(j == 0), stop=(j == 1))
        for half, src in ((0, ppool[:, 0:C]), (1, f2[:, 2 * s + 1, 0:C].bitcast(F32))):
            ex = e["p_scr"].tile([128, CPAD], F32, tag="scr")
            se = e["p_sm"].tile([128, 2], F32, tag="sm")
            nc.scalar.activation(ex[:, 0:C], src, AF.Exp, accum_out=se[:, 0:1])
            nc.vector.reciprocal(se[:, 1:2], se[:, 0:1])
            nc.vector.tensor_scalar_mul(ex[:, 0:C], ex[:, 0:C], se[:, 1:2])
            row0 = 256 * s + 128 * half
            nc.sync.dma_start(d["out_d"][row0:row0 + 128, :], ex[:, 0:C])


# ======================= host side =======================

_PROG_CACHE = {}


def _get_program(n_layers=L):
    if n_layers not in _PROG_CACHE:
        _PROG_CACHE[n_layers] = build_program(n_layers)
    return _PROG_CACHE[n_layers]


def make_in_maps(inputs, n_layers=L):
    """Build per-core input maps; fold LN scale/bias into weights, pre-cast
    weights to fp8/bf16."""
    f32 = lambda x: np.ascontiguousarray(np.asarray(x), dtype=np.float32)
    fp8 = lambda x: np.ascontiguousarray(
        np.asarray(x, dtype=np.float32).astype(ml_dtypes.float8_e4m3fn))
    bf16 = lambda x: np.ascontiguousarray(
        np.asarray(x, dtype=np.float32).astype(ml_dtypes.bfloat16))
    enc = np.asarray(inputs["encoded_batch"], dtype=np.int32)
    mask = np.asarray(inputs["mask"], dtype=np.int32)
    wpt = np.asarray(inputs["word_piece_tracked"], dtype=np.int32)

    Wq, Wk, Wv, Wo = (f32(inputs[k]) for k in ["Wq", "Wk", "Wv", "Wo"])
    Wi, Wo2 = f32(inputs["Wi"]), f32(inputs["Wo2"])
    bq, bk, bv, bo = (f32(inputs[k]) for k in ["bq", "bk", "bv", "bo"])
    bi, bo2 = f32(inputs["bi"]), f32(inputs["bo2"])
    ln1_s, ln1_b = f32(inputs["ln1_s"]), f32(inputs["ln1_b"])
    ln2_s, ln2_b = f32(inputs["ln2_s"]), f32(inputs["ln2_b"])

    s_in = np.empty((L, H), np.float32)
    b_in = np.empty((L, H), np.float32)
    s_in[0], b_in[0] = f32(inputs["emb_ln_s"]), f32(inputs["emb_ln_b"])
    s_in[1:], b_in[1:] = ln2_s[:L - 1], ln2_b[:L - 1]

    Wq8 = np.empty((L, H, H), ml_dtypes.float8_e4m3fn)
    Wk8 = np.empty_like(Wq8)
    Wv8 = np.empty_like(Wq8)
    Wi8 = np.empty((L, H, FF), ml_dtypes.float8_e4m3fn)
    bq_f = np.empty((L, H), np.float32)
    bk_f = np.empty_like(bq_f)
    y2b = np.empty_like(bq_f)
    bi_f = np.empty((L, FF), np.float32)
    DWv8 = np.empty_like(Wq8)
    DWi8 = np.empty_like(Wi8)
    for l in range(L):
        Wq8[l] = fp8(s_in[l][:, None] * Wq[l] * SW)
        Wk8[l] = fp8(s_in[l][:, None] * Wk[l] * SW)
        wv_t = s_in[l][:, None] * Wv[l] * SW
        Wv8[l] = fp8(wv_t)
        DWv8[l] = fp8(wv_t - Wv8[l].astype(np.float32))
        wi_t = ln1_s[l][:, None] * Wi[l] * SW
        Wi8[l] = fp8(wi_t)
        DWi8[l] = fp8(wi_t - Wi8[l].astype(np.float32))
        bq_f[l] = b_in[l] @ Wq[l] + bq[l]
        bk_f[l] = b_in[l] @ Wk[l] + bk[l]
        bv_full = b_in[l] @ Wv[l] + bv[l]
        bo_full = bv_full @ Wo[l] + bo[l]
        y2b[l] = b_in[l] + bo_full
        bi_f[l] = b_in_ff = ln1_b[l] @ Wi[l] + bi[l]

    # pooling matrix P[b, s, w] = 1/cnt[b,w] if seg[b,s]==w else 0
    cum = np.cumsum(wpt, axis=1)
    P = np.zeros((B, S, W), dtype=np.float32)
    for b in range(B):
        seg = np.searchsorted(cum[b], np.arange(S), side="right")
        valid = seg < W
        P[b, np.arange(S)[valid], seg[valid]] = 1.0 / wpt[b, seg[valid]]

    ab = (1.0 - mask.astype(np.float32)) * -10000.0

    rep = dict(
        word_emb=f32(inputs["word_emb"]),
        pos_p=f32(inputs["pos_emb"]) + f32(inputs["type_emb"])[0][None, :],
        Wq8=Wq8, Wk8=Wk8, Wv8=Wv8, DWv8=DWv8, Wob=bf16(Wo),
        Wi8=Wi8, DWi8=DWi8, Wo2b=bf16(Wo2),
        bq_f=bq_f, bk_f=bk_f, bi_f=bi_f,
        y2s=s_in, y2b=y2b,
        y1s=ln1_s, y1b=ln1_b + bo2,
        hs=ln2_s[n_layers - 1], hb=ln2_b[n_layers - 1],
        w1b=bf16(inputs["w1"]), b1=f32(inputs["b1"]),
        w2b=bf16(inputs["w2"]), b2=f32(inputs["b2"]),
        ones=np.ones((128, 128), np.float32),
    )

    in_maps = []
    for core in range(N_CORES):
        b0 = core * BPC
        m = dict(rep)
        m["enc"] = enc[b0:b0 + BPC].reshape(T, 1)
        m["ab"] = ab[b0:b0 + BPC].reshape(T)
        m["pmat"] = P[b0:b0 + BPC].reshape(T, W)
        in_maps.append(m)
    return in_maps


def kernel(**inputs):
    nc = _get_program(L)
    in_maps = make_in_maps(inputs, L)
    res = run_bass_kernel_spmd(nc, in_maps, core_ids=list(range(N_CORES)))
    out = np.concatenate([res.results[i]["out"].reshape(BPC, S, C)
                          for i in range(N_CORES)], axis=0)
    return out.astype(np.float32)


# revision 35
# speedup vs baseline: 1.3150x; 1.0162x over previous
"""Trainium2 Bass kernel for the CCG supertagger BERT model.

Data-parallel over batch: 16 samples -> 8 cores x 2 samples.
Key optimizations over the v1 kernel:
  - fp8e4m3 DoubleRow matmuls (0.5 cyc/row) for QKV, V, Wo, Wi, Wo2.
  - LayerNorm scale/bias folded into the next matmul's weights/biases on
    the host; residual re-applications via per-partition scalar ops.
  - LN stats broadcast across partitions by using a [128,128] ones
    stationary (cost only depends on moving rows), rstd computed as
    exp(-0.5*ln(var+eps)) on the scalar engine (stays in the exp table).
  - V projected directly token-major (no PE transposes); softmax sums
    folded into the ctx matmul via a 1/16-column appended to V.
  - All weights pre-cast to fp8/bf16 on the host (4x less HBM traffic).
"""
import numpy as np
import ml_dtypes

import concourse.bass as bass
import concourse.tile as tile
from concourse import bacc, mybir
from concourse.bass_utils import run_bass_kernel_spmd
from concourse.masks import make_identity

F32 = mybir.dt.float32
F32R = mybir.dt.float32r
BF16 = mybir.dt.bfloat16
FP8 = mybir.dt.float8e4
I32 = mybir.dt.int32
AF = mybir.ActivationFunctionType
ALU = mybir.AluOpType
PM = mybir.MatmulPerfMode

B, S, W = 16, 256, 128
V, H, L, NH, DH, FF, C = 30522, 768, 12, 12, 64, 3072, 425
EPS = 1e-12
N_CORES = 8
BPC = B // N_CORES          # samples per core
T = BPC * S                 # tokens per core (512)
HC = H // 128               # 6
FFC = FF // 128             # 24
TC = T // 128               # 4 token chunks
M1 = 1024
M1C = M1 // 128             # 8
CPAD = 448                  # padded class dim for sbuf tiles
SX = 16.0                   # fp8 activation scale
SW = 64.0                   # fp8 weight scale
ISC = 1.0 / (SX * SW)       # psum descale for fp8 x fp8 matmuls
VCOL = 1.0                  # value of the ones-column appended to V


def build_program(n_layers=L):
    nc = bacc.Bacc("TRN2", target_bir_lowering=False, debug=False,
                   num_devices=N_CORES)
    try:
        # Route Ln to the natural_log_exp table (which also serves Exp) so
        # each LayerNorm costs at most one activation-table switch.
        from concourse.hw_specs import get_activation_tables
        tabs = get_activation_tables(nc.m.arch)
        if "natural_log_exp_and_others" in tabs and "natural_log" in tabs:
            tabs["natural_log"].clear()
    except Exception:
        pass

    dt_ = lambda name, shape, dt, kind: nc.dram_tensor(name, shape, dt, kind=kind).ap()
    d = {}
    # per-core sharded inputs
    d["enc"] = dt_("enc", [T, 1], I32, "ExternalInput")
    d["ab"] = dt_("ab", [T], F32, "ExternalInput")          # attn bias per key pos
    d["pmat"] = dt_("pmat", [T, 128], F32, "ExternalInput")  # pooling matrices
    # replicated model inputs (host pre-processed)
    d["word_emb"] = dt_("word_emb", [V, H], F32, "ExternalInput")
    d["pos_p"] = dt_("pos_p", [S, H], F32, "ExternalInput")  # pos + type emb
    d["Wq8"] = dt_("Wq8", [L, H, H], FP8, "ExternalInput")
    d["Wk8"] = dt_("Wk8", [L, H, H], FP8, "ExternalInput")
    d["Wv8"] = dt_("Wv8", [L, H, H], FP8, "ExternalInput")
    d["DWv8"] = dt_("DWv8", [L, H, H], FP8, "ExternalInput")
    d["DWi8"] = dt_("DWi8", [L, H, FF], FP8, "ExternalInput")
    d["Wob"] = dt_("Wob", [L, H, H], BF16, "ExternalInput")
    d["Wi8"] = dt_("Wi8", [L, H, FF], FP8, "ExternalInput")
    d["Wo2b"] = dt_("Wo2b", [L, FF, H], BF16, "ExternalInput")
    d["bq_f"] = dt_("bq_f", [L, H], F32, "ExternalInput")
    d["bk_f"] = dt_("bk_f", [L, H], F32, "ExternalInput")
    d["bi_f"] = dt_("bi_f", [L, FF], F32, "ExternalInput")
    d["y2s"] = dt_("y2s", [L, H], F32, "ExternalInput")   # s_in/SX
    d["y2b"] = dt_("y2b", [L, H], F32, "ExternalInput")   # b_in + bo_full
    d["y1s"] = dt_("y1s", [L, H], F32, "ExternalInput")   # ln1_s/SX
    d["y1b"] = dt_("y1b", [L, H], F32, "ExternalInput")   # ln1_b + bo2
    d["hs"] = dt_("hs", [H], F32, "ExternalInput")        # ln2_s[-1]/SX
    d["hb"] = dt_("hb", [H], F32, "ExternalInput")        # ln2_b[-1]
    d["w1b"] = dt_("w1b", [H, M1], BF16, "ExternalInput")
    d["b1"] = dt_("b1", [M1], F32, "ExternalInput")
    d["w2b"] = dt_("w2b", [M1, C], BF16, "ExternalInput")
    d["b2"] = dt_("b2", [C], F32, "ExternalInput")
    d["ones"] = dt_("ones", [128, 128], F32, "ExternalInput")
    d["out_d"] = dt_("out", [T, C], F32, "ExternalOutput")

    with tile.TileContext(nc) as tc:
        from contextlib import ExitStack
        with ExitStack() as ctx:
            _emit(nc, tc, n_layers, d, ctx)
    nc.compile()
    return nc


def _emit(nc, tc, n_layers, d, ctx):
    pool = lambda name, bufs, space="SBUF": ctx.enter_context(
        tc.tile_pool(name=name, bufs=bufs, space=space))

    e = {}
    e["p_x8"] = pool("x8", 3)       # [128, HC, T] fp8 normalized acts (xhat*SX)
    e["p_xt"] = pool("xt", 2)       # [128, HC, T] f32r residual stream
    e["p_qk"] = pool("qk", 2)       # [128, HC, T] bf16 (QT, KT)
    e["p_v"] = pool("v", 1)         # [128, TC, 780] fp8 token-major V + ones cols
    e["p_c8"] = pool("c8", 2)       # [128, TC, H] bf16 ctok / [128, HC, T] fp8 ctxT
    e["p_exp"] = pool("exp", 3)     # [128, 256] bf16 exp tiles
    e["p_big"] = pool("big", 1)     # 12KB arena: emb x0 f32 / gel8 fp8
    e["p_scr"] = pool("scr", 3)     # [128, T] f32 scratch (t2, y2', y1')
    e["p_st"] = pool("st", 3)       # [128, T] f32 LN stats (musq/var/lnv/rstd)
    e["p_sq"] = pool("sq", 2)       # [128, T] f32r square scratch
    e["p_w4"] = pool("w4", 2)       # [128, HC, 2H] fp8 Wq|Wk
    e["p_wv"] = pool("wv", 1)       # [128, HC, 2H] fp8 Wv|DWv
    e["p_wo"] = pool("wo", 1)       # [128, HC, H] bf16 Wo
    e["p_wi"] = pool("wi", 1)       # [128, HC, FF] fp8 (Wi|DWi half)
    e["p_wo2"] = pool("wo2", 1)     # [128, FFC/2, H] bf16 Wo2 half
    e["p_w12"] = pool("w12", 1)     # [128, HC, M1] fp8 w1
    e["p_w2h"] = pool("w2h", 1)     # [128, M1C, C] fp8 w2
    e["p_f1"] = pool("f1", 1)       # [128, M1C, T] fp8
    e["p_f2"] = pool("f2", 1)       # [128, TC, CPAD] f32r
    e["p_pos"] = pool("pos", 1)     # [128, 2, H] bf16 pos embedding
    e["p_bias"] = pool("bias", 6)   # small per-layer bias tiles
    e["p_sm"] = pool("sm", 4)       # small scalars
    e["p_cst"] = pool("cst", 1)     # constants
    e["p_lnbc"] = pool("lnbc", 1)   # [128, C] bf16 b2 broadcast

    e["ps_mm"] = pool("ps_mm", 4, "PSUM")   # [128, 512] f32 main matmul banks
    e["ps_sc"] = pool("ps_sc", 4, "PSUM")   # scores / ctx / LN stat banks
    e["ps_st"] = e["ps_sc"]
    e["ps_cx"] = e["ps_sc"]

    # ---- constants ----
    identb = e["p_cst"].tile([128, 128], BF16, tag="identb")
    make_identity(nc, identb[:])
    ones128 = e["p_cst"].tile([128, 128], F32R, tag="ones128")
    nc.sync.dma_start(ones128[:], d["ones"].bitcast(F32R))
    eps_t = e["p_cst"].tile([128, 1], F32, tag="eps")
    nc.gpsimd.memset(eps_t[:], EPS)
    e["identb"] = identb
    e["ones128"], e["eps_t"] = ones128, eps_t

    ab_t = e["p_cst"].tile([128, TC], F32, tag="ab")
    nc.sync.dma_start(ab_t[:], d["ab"].rearrange("(c p) -> p c", p=128))
    e["ab_t"] = ab_t

    def colvec(ap_1d, n, tag, pool="p_bias"):
        t = e[pool].tile([128, n], F32, tag="bias")
        nc.sync.dma_start(t[:], ap_1d.rearrange("(c p) -> p c", p=128))
        return t
    e["colvec"] = colvec

    # =============== embedding ===============
    x0 = e["p_big"].tile([128, TC, H], F32, tag="big")
    for c in range(TC):
        idx_t = e["p_sm"].tile([128, 1], I32, tag="sm")
        nc.sync.dma_start(idx_t[:], d["enc"][128 * c:128 * (c + 1), :])
        nc.gpsimd.indirect_dma_start(
            out=x0[:, c, :], out_offset=None, in_=d["word_emb"][:],
            in_offset=bass.IndirectOffsetOnAxis(ap=idx_t[:, :1], axis=0))
    pos_t = e["p_pos"].tile([128, 2, H], BF16, tag="pos")
    nc.gpsimd.dma_start(pos_t[:, 0, :], d["pos_p"][0:128, :])
    nc.gpsimd.dma_start(pos_t[:, 1, :], d["pos_p"][128:256, :])

    xq8 = e["p_x8"].tile([128, HC, T], FP8, tag="x8")
    xh = e["p_xt"].tile([128, HC, T], F32R, tag="xt")
    for c in range(TC):
        xc = x0[:, c, :]
        nc.vector.tensor_tensor(out=xc, in0=xc, in1=pos_t[:, c % 2, :], op=ALU.add)
        # token-major layernorm (per-partition stats)
        su = e["p_sm"].tile([128, 4], F32, tag="sm")
        nc.vector.reduce_sum(out=su[:, 0:1], in_=xc, axis=mybir.AxisListType.X)
        sq = e["p_scr"].tile([128, H], F32, tag="scr")
        nc.scalar.activation(sq[:], xc, AF.Square, accum_out=su[:, 1:2])
        st = e["p_sm"].tile([128, 4], F32, tag="sm")
        nc.vector.tensor_scalar_mul(st[:, 0:1], su[:, 0:1], 1.0 / H)      # mu
        nc.vector.tensor_scalar_mul(st[:, 1:2], su[:, 1:2], 1.0 / H)      # m2
        nc.vector.tensor_tensor(out=st[:, 2:3], in0=st[:, 0:1], in1=st[:, 0:1], op=ALU.mult)
        nc.vector.tensor_tensor(out=st[:, 3:4], in0=st[:, 1:2], in1=st[:, 2:3], op=ALU.subtract)
        sd = e["p_sm"].tile([128, 2], F32, tag="sm")
        nc.scalar.activation(sd[:, 0:1], st[:, 3:4], AF.Ln, bias=eps_t[:, 0:1])
        nc.scalar.activation(sd[:, 1:2], sd[:, 0:1], AF.Exp, scale=-0.5)
        xcr = e["p_scr"].tile([128, H], BF16, tag="scr")
        nc.vector.tensor_scalar(out=xcr[:], in0=xc, scalar1=st[:, 0:1], scalar2=sd[:, 1:2],
                                op0=ALU.subtract, op1=ALU.mult)
        for k in range(HC):
            pt = e["ps_mm"].tile([128, 128], BF16, tag="ps_mm")
            nc.tensor.transpose(pt[:], xcr[:, 128 * k:128 * (k + 1)], identb[:])
            nc.scalar.activation(xh[:, k, 128 * c:128 * (c + 1)], pt[:], AF.Copy)
            nc.vector.tensor_scalar_mul(xq8[:, k, 128 * c:128 * (c + 1)],
                                        xh[:, k, 128 * c:128 * (c + 1)].bitcast(F32),
                                        SX)

    # =============== transformer layers ===============
    for l in range(n_layers):
        xq8, xh = _layer(nc, d, l, xq8, xh, e)

    # =============== head ===============
    _head(nc, d, xh, e)


def _wload(nc, dst_ap, src2d):
    """DMA a [K, F] dram weight into a k-major [128, K/128, F] SBUF AP."""
    nc.gpsimd.dma_start(dst_ap, src2d.rearrange("(k p) f -> p k f", p=128))


def _ln_finish(nc, e, A, mean_ps, sq_ps):
    """Partition-dim LN from broadcast stat psums; returns fp8 xhat*SX tile."""
    musq = e["p_st"].tile([128, T], F32, tag="st")
    nc.scalar.activation(musq[:], mean_ps[:], AF.Square, scale=1.0 / H)
    var = e["p_st"].tile([128, T], F32, tag="st")
    nc.vector.scalar_tensor_tensor(out=var[:], in0=sq_ps[:], scalar=1.0 / H,
                                   in1=musq[:], op0=ALU.mult, op1=ALU.subtract)
    lnv = e["p_st"].tile([128, T], F32, tag="st")
    nc.scalar.activation(lnv[:], var[:], AF.Ln, bias=e["eps_t"][:, 0:1])
    rstd = e["p_st"].tile([128, T], F32, tag="st")
    nc.scalar.activation(rstd[:], lnv[:], AF.Exp, scale=-0.5)
    xq8n = e["p_x8"].tile([128, HC, T], FP8, tag="x8")
    # fp8 outputs first (consumers wait on these); xhat writes lag one chunk
    lag = []
    for k in range(HC):
        t2 = e["p_scr"].tile([128, T], F32, tag="scr")
        nc.vector.scalar_tensor_tensor(out=t2[:], in0=mean_ps[:], scalar=-1.0 / H,
                                       in1=A[:, k, :].bitcast(F32),
                                       op0=ALU.mult, op1=ALU.add)
        nc.vector.scalar_tensor_tensor(out=xq8n[:, k, :], in0=t2[:], scalar=SX,
                                       in1=rstd[:], op0=ALU.mult, op1=ALU.mult)
        lag.append((k, t2))
        if len(lag) > 1:
            pk, pt2 = lag.pop(0)
            nc.vector.tensor_tensor(out=A[:, pk, :], in0=pt2[:], in1=rstd[:],
                                    op=ALU.mult)
    for pk, pt2 in lag:
        nc.vector.tensor_tensor(out=A[:, pk, :], in0=pt2[:], in1=rstd[:],
                                op=ALU.mult)
    return xq8n


def _layer(nc, d, l, xq8, xh_in, e):
    ps_mm, ps_st, ps_sc, ps_cx = e["ps_mm"], e["ps_st"], e["ps_sc"], e["ps_cx"]
    ones128, ab_t = e["ones128"], e["ab_t"]
    colvec = e["colvec"]

    # ---- per-layer weights & biases ----
    wqk = e["p_w4"].tile([128, HC, 2 * H], FP8, tag="w4")
    for i, wn in enumerate(["Wq8", "Wk8"]):
        _wload(nc, wqk[:, :, H * i:H * (i + 1)], d[wn][l])
    wv = e["p_wv"].tile([128, HC, 2 * H], FP8, tag="wv")
    _wload(nc, wv[:, :, 0:H], d["Wv8"][l])
    _wload(nc, wv[:, :, H:2 * H], d["DWv8"][l])
    wo = e["p_wo"].tile([128, HC, H], BF16, tag="wo")
    _wload(nc, wo[:], d["Wob"][l])
    bqk = e["p_bias"].tile([128, 2 * HC], F32, tag="bias")
    nc.sync.dma_start(bqk[:, 0:HC], d["bq_f"][l].rearrange("(c p) -> p c", p=128))
    nc.sync.dma_start(bqk[:, HC:2 * HC], d["bk_f"][l].rearrange("(c p) -> p c", p=128))
    yv = e["p_bias"].tile([128, 4 * HC], F32, tag="bias")
    for i, yn in enumerate(["y2s", "y2b", "y1s", "y1b"]):
        nc.sync.dma_start(yv[:, HC * i:HC * (i + 1)],
                          d[yn][l].rearrange("(c p) -> p c", p=128))
    bi_t = colvec(d["bi_f"][l], FFC, "bi")

    # ---- Q, K projections (dh-major) ----
    QT = e["p_qk"].tile([128, HC, T], BF16, tag="qk")
    KT = e["p_qk"].tile([128, HC, T], BF16, tag="qk")
    for pi, dst in enumerate([QT, KT]):
        for m in range(HC):
            pm_ = ps_mm.tile([128, T], F32, tag="ps_mm")
            for j in range(HC // 2):
                nc.tensor.matmul(
                    pm_[:], wqk[:, 2 * j:2 * j + 2, H * pi + 128 * m:H * pi + 128 * (m + 1)],
                    xq8[:, 2 * j:2 * j + 2, :],
                    start=(j == 0), stop=(j == HC // 2 - 1), perf_mode=PM.DoubleRow)
            nc.scalar.activation(dst[:, m, :], pm_[:], AF.Identity, scale=ISC,
                                 bias=bqk[:, HC * pi + m:HC * pi + m + 1])

    # ---- V projection, token-major, with 1/SX columns for softmax sums ----
    Vt = e["p_v"].tile([128, TC, 65 * NH], FP8, tag="v")
    vones = Vt[:].rearrange("p c (h x) -> p c h x", h=NH)[:, :, :, 64:65]
    nc.gpsimd.memset(vones, VCOL)
    for c in range(TC):
        for hf in range(2):
            pm_ = ps_mm.tile([128, 384], F32, tag="ps_mm")
            for dw in range(2):
                for j in range(HC // 2):
                    nc.tensor.matmul(
                        pm_[:], xq8[:, 2 * j:2 * j + 2, 128 * c:128 * (c + 1)],
                        wv[:, 2 * j:2 * j + 2, H * dw + 384 * hf:H * dw + 384 * (hf + 1)],
                        start=(dw == 0 and j == 0),
                        stop=(dw == 1 and j == HC // 2 - 1), perf_mode=PM.DoubleRow)
            dstv = Vt[:, c, 390 * hf:390 * (hf + 1)].rearrange(
                "p (h x) -> p h x", h=6)[:, :, 0:64]
            nc.scalar.activation(dstv, pm_[:].rearrange("p (h x) -> p h x", h=6),
                                 AF.Copy, scale=ISC)

    # ---- attention ----
    ctok = e["p_c8"].tile([128, TC, H], BF16, tag="c8")
    pend = []

    def emit_ctx(s, h, exp8):
        for i in range(2):
            pcx = ps_cx.tile([128, 65], F32, tag="ps_sc")
            nc.tensor.matmul(pcx[:], exp8[:, 0:2, 128 * i:128 * (i + 1)],
                             Vt[:, 2 * s:2 * s + 2, 65 * h:65 * h + 65],
                             start=True, stop=True, perf_mode=PM.DoubleRow)
            rec = e["p_sm"].tile([128, 1], F32, tag="sm")
            nc.vector.reciprocal(rec[:], pcx[:, 64:65])
            nc.vector.tensor_scalar_mul(
                ctok[:, 2 * s + i, 64 * h:64 * h + 64], pcx[:, 0:64], rec[:])

    for s in range(BPC):
        for h in range(NH):
            kc, po = h // 2, 64 * (h % 2)
            exp8 = e["p_exp"].tile([128, 2, 256], FP8, tag="exp")
            for j in range(2):
                psc = ps_sc.tile([128, 256], F32, tag="ps_sc")
                nc.tensor.matmul(
                    psc[:],
                    KT[po:po + 64, kc, 256 * s + 128 * j:256 * s + 128 * (j + 1)],
                    QT[po:po + 64, kc, 256 * s:256 * (s + 1)],
                    start=True, stop=True)
                nc.scalar.activation(exp8[:, j, :], psc[:], AF.Exp, scale=0.125,
                                     bias=ab_t[:, 2 * s + j:2 * s + j + 1])
            pend.append((s, h, exp8))
            if len(pend) > 1:
                emit_ctx(*pend.pop(0))
    while pend:
        emit_ctx(*pend.pop(0))

    # ---- transpose ctx to [H, T] bf16 ----
    ctxT = e["p_c8"].tile([128, HC, T], BF16, tag="c8")
    for c in range(TC):
        for k in range(HC):
            pt = ps_mm.tile([128, 128], BF16, tag="ps_mm")
            nc.tensor.transpose(pt[:], ctok[:, c, 128 * k:128 * (k + 1)],
                                e["identb"][:])
            nc.scalar.activation(ctxT[:, k, 128 * c:128 * (c + 1)], pt[:], AF.Copy)

    # ---- O projection + residual + LN1 stats ----
    A = e["p_xt"].tile([128, HC, T], F32R, tag="xt")
    mean_ps = ps_st.tile([128, T], F32, tag="ps_sc")
    sq_ps = ps_st.tile([128, T], F32, tag="ps_sc")
    sq_q = []
    for o in range(HC):
        pm_ = ps_mm.tile([128, T], F32, tag="ps_mm")
        for k in range(HC):
            nc.tensor.matmul(pm_[:], wo[:, k, 128 * o:128 * (o + 1)],
                             ctxT[:, k, :], start=(k == 0), stop=(k == HC - 1))
        y2o = e["p_scr"].tile([128, T], F32, tag="scr")
        nc.scalar.activation(y2o[:], xh_in[:, o, :].bitcast(F32), AF.Identity,
                             scale=yv[:, o:o + 1], bias=yv[:, HC + o:HC + o + 1])
        nc.vector.scalar_tensor_tensor(out=A[:, o, :], in0=pm_[:], scalar=1.0,
                                       in1=y2o[:], op0=ALU.mult, op1=ALU.add)
        sq = e["p_sq"].tile([128, T], F32R, tag="sq")
        nc.scalar.activation(sq[:], A[:, o, :].bitcast(F32), AF.Square)
        sq_q.append((o, sq))
        if o > 0:
            nc.tensor.matmul(mean_ps[:], ones128[:], A[:, o - 1, :],
                             start=(o - 1 == 0), stop=False)
            po, psq = sq_q.pop(0)
            nc.tensor.matmul(sq_ps[:], ones128[:], psq[:],
                             start=(po == 0), stop=False)
    nc.tensor.matmul(mean_ps[:], ones128[:], A[:, HC - 1, :],
                     start=False, stop=True)
    po, psq = sq_q.pop(0)
    nc.tensor.matmul(sq_ps[:], ones128[:], psq[:], start=False, stop=True)
    xq81 = _ln_finish(nc, e, A, mean_ps, sq_ps)


    # ---- FFN: two FF-chunk halves; Wi fp8+delta compensated, Wo2 bf16 ----
    Apre = e["p_xt"].tile([128, HC, T], F32R, tag="xt")
    mean2 = ps_st.tile([128, T], F32, tag="ps_sc")
    sq2 = ps_st.tile([128, T], F32, tag="ps_sc")
    FH = FF // 2                       # 1536 ff cols per half
    KH = FFC // 2                      # 12 ff chunks per half
    sq_q = []
    for wh in range(2):
        wi = e["p_wi"].tile([128, HC, FF], FP8, tag="wi")
        nc.gpsimd.dma_start(
            wi[:, :, 0:FH], d["Wi8"][l].rearrange("(k p) f -> p k f", p=128)
            [:, :, FH * wh:FH * (wh + 1)])
        nc.gpsimd.dma_start(
            wi[:, :, FH:FF], d["DWi8"][l].rearrange("(k p) f -> p k f", p=128)
            [:, :, FH * wh:FH * (wh + 1)])
        wo2 = e["p_wo2"].tile([128, KH, H], BF16, tag="wo2")
        _wload(nc, wo2[:], d["Wo2b"][l, FH * wh:FH * (wh + 1)])
        gel = e["p_big"].tile([128, KH, T], BF16, tag="big")
        for mi in range(KH):
            m = KH * wh + mi
            pm_ = ps_mm.tile([128, T], F32, tag="ps_mm")
            for dw in range(2):
                for j in range(HC // 2):
                    nc.tensor.matmul(
                        pm_[:], wi[:, 2 * j:2 * j + 2,
                                   FH * dw + 128 * mi:FH * dw + 128 * (mi + 1)],
                        xq81[:, 2 * j:2 * j + 2, :],
                        start=(dw == 0 and j == 0),
                        stop=(dw == 1 and j == HC // 2 - 1), perf_mode=PM.DoubleRow)
            nc.scalar.activation(gel[:, mi, :], pm_[:], AF.Gelu, scale=ISC,
                                 bias=bi_t[:, m:m + 1])
        for o in range(HC):
            pm_ = ps_mm.tile([128, T], F32, tag="ps_mm")
            for k in range(KH):
                nc.tensor.matmul(pm_[:], wo2[:, k, 128 * o:128 * (o + 1)],
                                 gel[:, k, :], start=(k == 0), stop=(k == KH - 1))
            if wh == 0:
                y1o = e["p_scr"].tile([128, T], F32, tag="scr")
                nc.scalar.activation(y1o[:], A[:, o, :].bitcast(F32), AF.Identity,
                                     scale=yv[:, 2 * HC + o:2 * HC + o + 1],
                                     bias=yv[:, 3 * HC + o:3 * HC + o + 1])
                nc.vector.scalar_tensor_tensor(out=Apre[:, o, :], in0=pm_[:],
                                               scalar=1.0, in1=y1o[:],
                                               op0=ALU.mult, op1=ALU.add)
            else:
                nc.vector.scalar_tensor_tensor(out=Apre[:, o, :], in0=pm_[:],
                                               scalar=1.0,
                                               in1=Apre[:, o, :].bitcast(F32),
                                               op0=ALU.mult, op1=ALU.add)
                sq = e["p_sq"].tile([128, T], F32R, tag="sq")
                nc.scalar.activation(sq[:], Apre[:, o, :].bitcast(F32), AF.Square)
                sq_q.append((o, sq))
                if o > 0:
                    nc.tensor.matmul(mean2[:], ones128[:], Apre[:, o - 1, :],
                                     start=(o - 1 == 0), stop=False)
                    po, psq = sq_q.pop(0)
                    nc.tensor.matmul(sq2[:], ones128[:], psq[:],
                                     start=(po == 0), stop=False)
    nc.tensor.matmul(mean2[:], ones128[:], Apre[:, HC - 1, :],
                     start=False, stop=True)
    po, psq = sq_q.pop(0)
    nc.tensor.matmul(sq2[:], ones128[:], psq[:], start=False, stop=True)
    return _ln_finish(nc, e, Apre, mean2, sq2), Apre


def _head(nc, d, xh, e):
    ps_mm = e["ps_mm"]
    colvec = e["colvec"]
    hs_t = colvec(d["hs"], HC, "hs")
    hb_t = colvec(d["hb"], HC, "hb")
    # relu(x) with final LN scale/bias fused, bf16
    reluT = e["p_qk"].tile([128, HC, T], BF16, tag="qk")
    for k in range(HC):
        nc.scalar.activation(reluT[:, k, :], xh[:, k, :].bitcast(F32), AF.Relu,
                             scale=hs_t[:, k:k + 1], bias=hb_t[:, k:k + 1])
    # f1 = relu(relu(x) @ w1 + b1), [M1C, T] bf16
    b1_t = colvec(d["b1"], M1C, "b1")
    w1t = e["p_w12"].tile([128, HC, M1], BF16, tag="w12")
    _wload(nc, w1t[:], d["w1b"])
    f1 = e["p_f1"].tile([128, M1C, T], BF16, tag="f1")
    for m in range(M1C):
        pm_ = ps_mm.tile([128, T], F32, tag="ps_mm")
        for k in range(HC):
            nc.tensor.matmul(pm_[:], w1t[:, k, 128 * m:128 * (m + 1)], reluT[:, k, :],
                             start=(k == 0), stop=(k == HC - 1))
        nc.scalar.activation(f1[:, m, :], pm_[:], AF.Relu, bias=b1_t[:, m:m + 1])
    # f2 = f1 @ w2 + b2, token-major [TC, C]
    w2t = e["p_w2h"].tile([128, M1C, C], BF16, tag="w2h")
    _wload(nc, w2t[:], d["w2b"])
    b2bc = e["p_lnbc"].tile([128, C], BF16, tag="lnbc")
    nc.gpsimd.dma_start(b2bc[:], d["b2"][None, :].partition_broadcast(128)[:, 0, :])
    f2 = e["p_f2"].tile([128, TC, CPAD], F32R, tag="f2")
    nc.gpsimd.memset(f2[:].bitcast(F32), 0.0)
    for c in range(TC):
        pm_ = ps_mm.tile([128, C], F32, tag="ps_mm")
        for k in range(M1C):
            nc.tensor.matmul(pm_[:], f1[:, k, 128 * c:128 * (c + 1)], w2t[:, k, :],
                             start=(k == 0), stop=(k == M1C - 1))
        nc.vector.tensor_tensor(out=f2[:, c, 0:C], in0=pm_[:], in1=b2bc[:],
                                op=ALU.add)
    # pooling + final softmax (N padded to 428 for fp32r)
    CP2 = 428
    for s in range(BPC):
        ppool = ps_mm.tile([128, CP2], F32, tag="ps_mm")
        for j in range(2):
            pm_t = e["p_exp"].tile([128, 128], F32R, tag="exp")
            nc.sync.dma_start(pm_t[:], d["pmat"][256 * s + 128 * j:256 * s + 128 * (j + 1), :].bitcast(F32R))
            nc.tensor.matmul(ppool[:], pm_t[:], f2[:, 2 * s + j, 0:CP2],
                             start=(j == 0), stop=(j == 1))
        for half, src in ((0, ppool[:, 0:C]), (1, f2[:, 2 * s + 1, 0:C].bitcast(F32))):
            ex = e["p_scr"].tile([128, CPAD], F32, tag="scr")
            se = e["p_sm"].tile([128, 2], F32, tag="sm")
            nc.scalar.activation(ex[:, 0:C], src, AF.Exp, accum_out=se[:, 0:1])
            nc.vector.reciprocal(se[:, 1:2], se[:, 0:1])
            nc.vector.tensor_scalar_mul(ex[:, 0:C], ex[:, 0:C], se[:, 1:2])
            row0 = 256 * s + 128 * half
            nc.sync.dma_start(d["out_d"][row0:row0 + 128, :], ex[:, 0:C])


# ======================= host side =======================

_PROG_CACHE = {}


def _get_program(n_layers=L):
    if n_layers not in _PROG_CACHE:
        _PROG_CACHE[n_layers] = build_program(n_layers)
    return _PROG_CACHE[n_layers]


def make_in_maps(inputs, n_layers=L):
    """Build per-core input maps; fold LN scale/bias into weights, pre-cast
    weights to fp8/bf16."""
    f32 = lambda x: np.ascontiguousarray(np.asarray(x), dtype=np.float32)
    fp8 = lambda x: np.ascontiguousarray(
        np.asarray(x, dtype=np.float32).astype(ml_dtypes.float8_e4m3fn))
    bf16 = lambda x: np.ascontiguousarray(
        np.asarray(x, dtype=np.float32).astype(ml_dtypes.bfloat16))
    enc = np.asarray(inputs["encoded_batch"], dtype=np.int32)
    mask = np.asarray(inputs["mask"], dtype=np.int32)
    wpt = np.asarray(inputs["word_piece_tracked"], dtype=np.int32)

    Wq, Wk, Wv, Wo = (f32(inputs[k]) for k in ["Wq", "Wk", "Wv", "Wo"])
    Wi, Wo2 = f32(inputs["Wi"]), f32(inputs["Wo2"])
    bq, bk, bv, bo = (f32(inputs[k]) for k in ["bq", "bk", "bv", "bo"])
    bi, bo2 = f32(inputs["bi"]), f32(inputs["bo2"])
    ln1_s, ln1_b = f32(inputs["ln1_s"]), f32(inputs["ln1_b"])
    ln2_s, ln2_b = f32(inputs["ln2_s"]), f32(inputs["ln2_b"])

    s_in = np.empty((L, H), np.float32)
    b_in = np.empty((L, H), np.float32)
    s_in[0], b_in[0] = f32(inputs["emb_ln_s"]), f32(inputs["emb_ln_b"])
    s_in[1:], b_in[1:] = ln2_s[:L - 1], ln2_b[:L - 1]

    Wq8 = np.empty((L, H, H), ml_dtypes.float8_e4m3fn)
    Wk8 = np.empty_like(Wq8)
    Wv8 = np.empty_like(Wq8)
    Wi8 = np.empty((L, H, FF), ml_dtypes.float8_e4m3fn)
    bq_f = np.empty((L, H), np.float32)
    bk_f = np.empty_like(bq_f)
    y2b = np.empty_like(bq_f)
    bi_f = np.empty((L, FF), np.float32)
    DWv8 = np.empty_like(Wq8)
    DWi8 = np.empty_like(Wi8)
    for l in range(L):
        Wq8[l] = fp8(s_in[l][:, None] * Wq[l] * SW)
        Wk8[l] = fp8(s_in[l][:, None] * Wk[l] * SW)
        wv_t = s_in[l][:, None] * Wv[l] * SW
        Wv8[l] = fp8(wv_t)
        DWv8[l] = fp8(wv_t - Wv8[l].astype(np.float32))
        wi_t = ln1_s[l][:, None] * Wi[l] * SW
        Wi8[l] = fp8(wi_t)
        DWi8[l] = fp8(wi_t - Wi8[l].astype(np.float32))
        bq_f[l] = b_in[l] @ Wq[l] + bq[l]
        bk_f[l] = b_in[l] @ Wk[l] + bk[l]
        bv_full = b_in[l] @ Wv[l] + bv[l]
        bo_full = bv_full @ Wo[l] + bo[l]
        y2b[l] = b_in[l] + bo_full
        bi_f[l] = b_in_ff = ln1_b[l] @ Wi[l] + bi[l]

    # pooling matrix P[b, s, w] = 1/cnt[b,w] if seg[b,s]==w else 0
    cum = np.cumsum(wpt, axis=1)
    P = np.zeros((B, S, W), dtype=np.float32)
    for b in range(B):
        seg = np.searchsorted(cum[b], np.arange(S), side="right")
        valid = seg < W
        P[b, np.arange(S)[valid], seg[valid]] = 1.0 / wpt[b, seg[valid]]

    ab = (1.0 - mask.astype(np.float32)) * -10000.0

    rep = dict(
        word_emb=f32(inputs["word_emb"]),
        pos_p=f32(inputs["pos_emb"]) + f32(inputs["type_emb"])[0][None, :],
        Wq8=Wq8, Wk8=Wk8, Wv8=Wv8, DWv8=DWv8, Wob=bf16(Wo),
        Wi8=Wi8, DWi8=DWi8, Wo2b=bf16(Wo2),
        bq_f=bq_f, bk_f=bk_f, bi_f=bi_f,
        y2s=s_in, y2b=y2b,
        y1s=ln1_s, y1b=ln1_b + bo2,
        hs=ln2_s[n_layers - 1], hb=ln2_b[n_layers - 1],
        w1b=bf16(inputs["w1"]), b1=f32(inputs["b1"]),
        w2b=bf16(inputs["w2"]), b2=f32(inputs["b2"]),
        ones=np.ones((128, 128), np.float32),
    )

    in_maps = []
    for core in range(N_CORES):
        b0 = core * BPC
        m = dict(rep)
        m["enc"] = enc[b0:b0 + BPC].reshape(T, 1)
        m["ab"] = ab[b0:b0 + BPC].reshape(T)
        m["pmat"] = P[b0:b0 + BPC].reshape(T, W)
        in_maps.append(m)
    return in_maps


def kernel(**inputs):
    nc = _get_program(L)
    in_maps = make_in_maps(inputs, L)
    res = run_bass_kernel_spmd(nc, in_maps, core_ids=list(range(N_CORES)))
    out = np.concatenate([res.results[i]["out"].reshape(BPC, S, C)
                          for i in range(N_CORES)], axis=0)
    return out.astype(np.float32)
